# revision 38
# baseline (speedup 1.0000x reference)
"""Trainium2 Bass kernel for CenterGeoAttention (N=65536, D=1024, H=16).

Strategy (row-shard N across 8 cores, fp8 DoubleRow compute):

  - All heavy matmuls run in fp8e4m3 with MatmulPerfMode.DoubleRow
    (K=256 per instruction, 2 multiplies/cycle) against an SBUF-resident
    fp8 copy of the h shard (h8_s, 8 MiB).  Weights are pre-scaled by 64
    on the host so their N(0, 0.02) entries sit in fp8's normal range;
    the 1/64 descale folds into activation-scale / scalar_tensor_tensor.
  - LayerNorm is folded into rank-1 corrections (as before): the
    logits sweep's stationary operand carries [64*Wkp | ones] so row 16
    of the output is the per-row sum (mean) for free; the sumsq sweep
    uses fp8 squares computed on the Scalar engine.
  - The weighted V sum never materializes V: G = (p*r)^T @ h8 via fp8
    DoubleRow, AllReduce-add [G | PRM | S], then the tiny post-AR GEMV
    chain (out_center, h_c_new, a0, g0) runs on fp8 weights.
  - Pass 2: 3 big DR matmuls per chunk (h@W1t, h@Wgt, silu@W2) read the
    resident h8; fp32 h streams in only for the residual add.
  - Wv/Wo/W1b/Wgb prefetch during pass-1 chunks; W1t/Wgt/W2h load
    during the AllReduce so the inter-pass valley is just AR latency.
"""

import os
import ml_dtypes
import numpy as np

import concourse.bass as bass
import concourse.bacc as bacc
import concourse.tile as tile
import concourse.mybir as mybir
from concourse.bass_utils import run_bass_kernel_spmd

F32 = mybir.dt.float32
F8 = mybir.dt.float8e4
BF16 = mybir.dt.bfloat16
AF = mybir.ActivationFunctionType
OP = mybir.AluOpType
AX = mybir.AxisListType
DRM = mybir.MatmulPerfMode.DoubleRow

NCORES = 8
N, D, H, HD, BIAS = 65536, 1024, 16, 64, 128
NS = N // NCORES            # 8192 rows per core
CH = 512                    # row-chunk
NCH = NS // CH              # 16 chunks
KT = D // 128               # 8 feature tiles
EPS = 1e-5
RES = 0.5
SCL = 64.0                  # fp8 weight pre-scale

_CACHE = {}
LAST_RESULTS = None  # BassKernelResults from the most recent run (for test.py)


def _build(ncores=NCORES, variant="full", nch=NCH):
    nc = bacc.Bacc("TRN2", target_bir_lowering=False, debug=False,
                   num_devices=ncores)

    def din(name, shape, dt=F32):
        return nc.dram_tensor(name, list(shape), dt, kind="ExternalInput").ap()

    # per-core tensors
    hT = din("hT", (D, NS))               # h_shard^T fp32 (residual stream)
    h8T = din("h8T", (D, NS), F8)         # h_shard^T fp8
    hN8 = din("hN8", (NS, D), F8)         # h_shard natural fp8
    bT = din("bT", (BIAS, NS), BF16)      # bias_feat^T shard
    # shared weights
    Wkp8 = din("Wkp8", (D, 64), F8)       # [64*Wkp | pad | ones@32 | pad]
    Wb = din("Wb", (BIAS, H), BF16)       # 64*Wb
    W1t8 = din("W1t8", (D, D), F8)        # 64*W1[:D]
    Wgt8 = din("Wgt8", (D, D), F8)        # 64*Wg[:D]
    W2h8 = din("W2h8", (D, D), F8)        # 64*RES*W2
    Wv8 = din("Wv8", (D, D), F8)          # 64*Wv
    Wo8 = din("Wo8", (D, D), F8)          # 64*Wo
    W1b8 = din("W1b8", (D, D), F8)        # 64*W1[D:]
    Wgb8 = din("Wgb8", (D, D), F8)        # 64*Wg[D:]
    # small constants
    idn = din("idn", (128, 128), F32)
    ones16 = din("ones16", (128, KT * 16 * NCH), F8)  # block c = ones in col c
    ncg1 = din("ncg1", (1, H), BF16)      # 64*(-cg) as K=1 stationary
    eps16 = din("eps16", (16, 1), F32)
    cbv = din("cbv", (H, 1), F32)         # cb per head (exp bias)
    gb16 = din("gb16", (H, D), F32)       # gamma_a broadcast rows
    bb16 = din("bb16", (H, D), F32)       # beta_a broadcast rows
    hcvN = din("hcvN", (1, D), F32)       # h[c] natural row
    b1N = din("b1N", (1, D), F32)
    bgN = din("bgN", (1, D), F32)
    b2v = din("b2v", (128, KT), F32)      # 64*RES*b2

    outT = nc.dram_tensor("outT", [D, NS], F32, kind="ExternalOutput").ap()
    outC = nc.dram_tensor("outC", [1, D], F32, kind="ExternalOutput").ap()

    with tile.TileContext(nc) as tc:
        with (
            tc.tile_pool(name="persist", bufs=1) as pp,
            tc.tile_pool(name="dram", bufs=1, space="DRAM") as dram,
        ):
            # ---- long-lived small tiles ----
            idn_s = pp.tile([128, 128], F32, tag="idn")
            nc.sync.dma_start(out=idn_s[:], in_=idn[:])
            cbv_s = pp.tile([H, 1], F32, tag="cbv")
            nc.sync.dma_start(out=cbv_s[:], in_=cbv[:])
            b2v_s = pp.tile([128, KT], F32, tag="b2v")
            nc.sync.dma_start(out=b2v_s[:], in_=b2v[:])
            ones16_s = pp.tile([128, KT, 16 * NCH], F8, tag="ones16")
            nc.sync.dma_start(out=ones16_s[:], in_=ones16[:])
            ncg1_s = pp.tile([1, H], BF16, tag="ncg1")
            nc.sync.dma_start(out=ncg1_s[:], in_=ncg1[:])
            eps16_s = pp.tile([16, 1], F32, tag="eps16")
            nc.sync.dma_start(out=eps16_s[:], in_=eps16[:])
            Wkp8_s = pp.tile([128, KT, 64], F8, tag="Wkp8")
            for k in range(KT):
                nc.sync.dma_start(out=Wkp8_s[:, k:k + 1, :],
                                  in_=Wkp8[k * 128:(k + 1) * 128, :])
            Wb_s = pp.tile([BIAS, H], BF16, tag="Wb")
            nc.sync.dma_start(out=Wb_s[:], in_=Wb[:])

            h8_s = pp.tile([128, KT, NS], F8, tag="h8")
            Gacc = pp.tile([H, D], F32, tag="Gacc")
            sCols = pp.tile([H, NCH], F32, tag="sCols")
            g0_s = pp.tile([128, KT], F32, tag="g0")
            a0_s = pp.tile([128, KT], F32, tag="a0")
            GnT8 = pp.tile([128, KT, H], F8, tag="GnT8")
            ocv8 = pp.tile([128, KT, 16], F8, tag="ocv8")
            hcn8 = pp.tile([128, KT, 16], F8, tag="hcn8")

            # resident fp8 weights, loaded during pass 1 / the AR valley
            wres_cm = tc.tile_pool(name="wres", bufs=1)
            wres = wres_cm.__enter__()
            wv_s = wres.tile([128, KT, D], F8, tag="wv")
            wo_s = wres.tile([128, KT, D], F8, tag="wo")
            w1b_s = wres.tile([128, KT, D], F8, tag="w1b")
            wgb_s = wres.tile([128, KT, D], F8, tag="wgb")
            w1t_s = wres.tile([128, KT, D], F8, tag="w1t")
            wgt_s = wres.tile([128, KT, D], F8, tag="wgt")
            w2h_s = wres.tile([128, KT, D], F8, tag="w2h")
            PREFETCH = [(wv_s, Wv8), (wo_s, Wo8), (w1b_s, W1b8), (wgb_s, Wgb8)]
            VALLEY = [(w1t_s, W1t8), (wgt_s, Wgt8), (w2h_s, W2h8)]

            # =========================== PASS 1 ===========================
            # -- loop A: per-row sum and sum-of-squares, banked into a
            #    [16, CH] psum tile (chunk c -> row c via one-hot stationary)
            strips_cm = tc.tile_pool(name="strips", bufs=1)
            strips = strips_cm.__enter__()
            rb_all = strips.tile([1, NS], F32, tag="rb_all")  # 1/sd strip
            tm_all = strips.tile([1, NS], BF16, tag="tm_all")  # mean strip
            with (
                tc.tile_pool(name="pAsb", bufs=2) as sbA,
                tc.tile_pool(name="pAps", bufs=1, space="PSUM") as psS,
                tc.tile_pool(name="pAsb1", bufs=1) as sbM,
            ):
                SM16 = psS.tile([16, CH], F32, tag="SM16")
                SQ16 = psS.tile([16, CH], F32, tag="SQ16")
                for c in range(nch):
                    c0 = c * CH
                    nc.sync.dma_start(
                        out=h8_s[:, :, c0:c0 + CH],
                        in_=h8T[:, c0:c0 + CH].rearrange(
                            "(k p) j -> p k j", p=128))
                    if 1 <= c <= len(PREFETCH):
                        wsb, wd = PREFETCH[c - 1]
                        nc.scalar.dma_start(
                            out=wsb[:],
                            in_=wd[:].rearrange("(k p) j -> p k j", p=128))
                    sq8 = sbA.tile([128, KT, CH], F8, tag="sq8")
                    nc.vector.tensor_mul(sq8[:], h8_s[:, :, c0:c0 + CH],
                                         h8_s[:, :, c0:c0 + CH])
                    oc0 = c * 16
                    for kp in range(0, KT, 2):
                        nc.tensor.matmul(SM16[:],
                                         ones16_s[:, kp:kp + 2, oc0:oc0 + 16],
                                         h8_s[:, kp:kp + 2, c0:c0 + CH],
                                         start=(c == 0 and kp == 0),
                                         stop=(c == nch - 1 and kp == KT - 2),
                                         perf_mode=DRM)
                    for kp in range(0, KT, 2):
                        nc.tensor.matmul(SQ16[:],
                                         ones16_s[:, kp:kp + 2, oc0:oc0 + 16],
                                         sq8[:, kp:kp + 2, :],
                                         start=(c == 0 and kp == 0),
                                         stop=(c == nch - 1 and kp == KT - 2),
                                         perf_mode=DRM)
                # -- mid: batched LayerNorm stats for all 16 chunks at once
                tm16 = sbM.tile([16, CH], F32, tag="tm16")
                nc.vector.tensor_scalar_mul(tm16[:], SM16[:], 1.0 / D)
                msq16 = sbM.tile([16, CH], F32, tag="msq16")
                nc.vector.tensor_mul(msq16[:], tm16[:], tm16[:])
                var16 = sbM.tile([16, CH], F32, tag="var16")
                nc.vector.scalar_tensor_tensor(
                    var16[:], SQ16[:], 1.0 / D, msq16[:],
                    op0=OP.mult, op1=OP.subtract)
                sd16 = sbM.tile([16, CH], F32, tag="sd16")
                nc.scalar.activation(sd16[:], var16[:], AF.Sqrt,
                                     bias=eps16_s[:, 0:1])
                r16 = sbM.tile([16, CH], F32, tag="r16")
                nc.vector.reciprocal_approx_fast(r16[:], sd16[:])
                tmb16 = sbM.tile([16, CH], BF16, tag="tmb16")
                nc.vector.tensor_copy(tmb16[:], tm16[:])
                # reshape [16, CH] -> [1, NS] strips (row c -> cols c*CH...)
                nc.sync.dma_start(out=rb_all[:], in_=r16[:])
                nc.sync.dma_start(out=tm_all[:], in_=tmb16[:])

            # -- loop B: logits, softmax, and the G accumulation
            psG_cm = tc.tile_pool(name="psG", bufs=1, space="PSUM")
            psG = psG_cm.__enter__()
            G = psG.tile([H, D], F32, tag="G")
            with (
                tc.tile_pool(name="p1sb", bufs=1) as sb1,
                tc.tile_pool(name="p1sb2", bufs=2) as sb2,
                tc.tile_pool(name="p1psA", bufs=2, space="PSUM") as psA,
                tc.tile_pool(name="p1psB", bufs=1, space="PSUM") as psB,
            ):
                for c in range(nch):
                    c0 = c * CH
                    hN8c = sb2.tile([128, 4, D], F8, tag="hN8c")
                    nc.sync.dma_start(
                        out=hN8c[:],
                        in_=hN8[c0:c0 + CH, :].rearrange(
                            "(jj p) d -> p jj d", p=128))
                    bTc = sb2.tile([BIAS, CH], BF16, tag="bTc")
                    nc.sync.dma_start(out=bTc[:], in_=bT[:, c0:c0 + CH])

                    # Lp = 64*(Wkp^T h8 + ncg x m)  (ncg term via K=1 matmul)
                    Lp = psA.tile([H, CH], F32, tag="Lp")
                    for kp in range(0, KT, 2):
                        nc.tensor.matmul(Lp[:], Wkp8_s[:, kp:kp + 2, 0:16],
                                         h8_s[:, kp:kp + 2, c0:c0 + CH],
                                         start=(kp == 0), stop=False,
                                         perf_mode=DRM)
                    nc.tensor.matmul(Lp[:], ncg1_s[:],
                                     tm_all[:, c0:c0 + CH],
                                     start=False, stop=True)
                    L2 = psB.tile([H, CH], F32, tag="L2")
                    nc.tensor.matmul(L2[:], Wb_s[:], bTc[:],
                                     start=True, stop=True)

                    rb16 = sb2.tile([H, CH], F32, tag="rb16")
                    nc.gpsimd.partition_broadcast(rb16[:],
                                                  rb_all[:, c0:c0 + CH])
                    t3 = sb1.tile([H, CH], F32, tag="t3")
                    nc.vector.tensor_mul(t3[:], Lp[:], rb16[:])
                    t5 = sb2.tile([H, CH], F32, tag="t5")
                    nc.vector.tensor_add(t5[:], t3[:], L2[:])
                    pT = sb2.tile([H, CH], F32, tag="pT")
                    nc.scalar.activation(pT[:], t5[:], AF.Exp,
                                         bias=cbv_s[:, 0:1], scale=1.0 / SCL,
                                         accum_out=sCols[:, c:c + 1])
                    prT = sb2.tile([H, CH], F32, tag="prT")
                    nc.vector.tensor_mul(prT[:], pT[:], rb16[:])
                    # transpose p*r to natural fp8 and accumulate G
                    tp = psB.tile([128, 4 * H], F32, tag="tp")
                    for j in range(4):
                        nc.tensor.transpose(
                            tp[:, j * H:(j + 1) * H],
                            prT[:, j * 128:(j + 1) * 128],
                            idn_s[0:16, 0:16])
                    pr8 = sb2.tile([128, 4, H], F8, tag="pr8")
                    nc.vector.tensor_copy(pr8[:], tp[:])
                    for jp in (0, 2):
                        for half in range(2):
                            h0 = half * CH
                            nc.tensor.matmul(
                                G[:, h0:h0 + CH],
                                pr8[:, jp:jp + 2, :],
                                hN8c[:, jp:jp + 2, h0:h0 + CH],
                                start=(c == 0 and jp == 0),
                                stop=(c == nch - 1 and jp == 2),
                                perf_mode=DRM)
                nc.vector.tensor_copy(Gacc[:], G[:])
                if variant == "p1":
                    nc.sync.dma_start(out=outT[0:H, 0:D], in_=Gacc[:])
                    nc.sync.dma_start(out=outT[H:2 * H, 0:NCH], in_=sCols[:])
            strips_cm.__exit__(None, None, None)

            if variant != "p1":
                psG_cm.__exit__(None, None, None)
                # pass-2 weights stream on the ACT queue while the AR runs
                for wsb, wd in VALLEY:
                    nc.scalar.dma_start(
                        out=wsb[:],
                        in_=wd[:].rearrange("(k p) j -> p k j", p=128))
                # ---- local partials -> AllReduce ----
                # PRM = row-sum(G)/D exactly (sum_d G[h,d] = D * sum p*r*m)
                S16 = pp.tile([H, 1], F32, tag="S16")
                nc.vector.reduce_sum(S16[:], sCols[:], axis=AX.X)
                PRM16 = pp.tile([H, 1], F32, tag="PRM16")
                nc.vector.reduce_sum(PRM16[:], Gacc[:], axis=AX.X)
                nc.vector.tensor_scalar_mul(PRM16[:], PRM16[:], 1.0 / D)

                arin = dram.tile([H, D + 2], F32, tag="arin")
                arout = dram.tile([H, D + 2], F32, tag="arout")
                nc.sync.dma_start(out=arin[:, 0:D], in_=Gacc[:])
                nc.sync.dma_start(out=arin[:, D:D + 1], in_=PRM16[:])
                nc.sync.dma_start(out=arin[:, D + 1:D + 2], in_=S16[:])
                if variant == "nocc":
                    nc.sync.dma_start(out=arout[:], in_=arin[:])
                else:
                    nc.gpsimd.collective_compute(
                        "AllReduce", OP.add,
                        replica_groups=[list(range(ncores))],
                        ins=[arin.opt()], outs=[arout.opt()])
                # ---- G corrections + normalize + tiny GEMV chain ----
                with (
                    tc.tile_pool(name="postsb", bufs=1) as psb,
                    tc.tile_pool(name="postps", bufs=1, space="PSUM") as ps2,
                ):
                    hcv_s = psb.tile([1, D], F32, tag="hcv")
                    nc.sync.dma_start(out=hcv_s[:], in_=hcvN[:])
                    b1v_s = psb.tile([1, D], F32, tag="b1v")
                    nc.sync.dma_start(out=b1v_s[:], in_=b1N[:])
                    bgv_s = psb.tile([1, D], F32, tag="bgv")
                    nc.sync.dma_start(out=bgv_s[:], in_=bgN[:])
                    gb16_s = psb.tile([H, D], F32, tag="gb16")
                    nc.sync.dma_start(out=gb16_s[:], in_=gb16[:])
                    bb16_s = psb.tile([H, D], F32, tag="bb16")
                    nc.sync.dma_start(out=bb16_s[:], in_=bb16[:])
                    Gar = psb.tile([H, D], F32, tag="Gar")
                    nc.sync.dma_start(out=Gar[:], in_=arout[:, 0:D])
                    PSar = psb.tile([H, 2], F32, tag="PSar")
                    nc.sync.dma_start(out=PSar[:], in_=arout[:, D:D + 2])
                    Gn = psb.tile([H, D], F32, tag="Gn")
                    nc.vector.tensor_scalar_sub(Gn[:], Gar[:], PSar[:, 0:1])
                    nc.vector.tensor_mul(Gn[:], Gn[:], gb16_s[:])
                    nc.vector.scalar_tensor_tensor(
                        Gn[:], bb16_s[:], PSar[:, 1:2], Gn[:],
                        op0=OP.mult, op1=OP.add)
                    sr = psb.tile([H, 1], F32, tag="sr")
                    nc.vector.reciprocal(sr[:], PSar[:, 1:2])
                    nc.vector.tensor_scalar_mul(Gn[:], Gn[:], sr[:, 0:1])

                    tpg = ps2.tile([128, KT * H], F32, tag="tpg")
                    for m in range(KT):
                        nc.tensor.transpose(
                            tpg[:, m * H:(m + 1) * H],
                            Gn[:, m * 128:(m + 1) * 128],
                            idn_s[0:16, 0:16])
                    nc.vector.tensor_copy(GnT8[:], tpg[:])

                    # out_center natural: Gn row h dot Wv columns (weights
                    # move, Gn^T stationary) -> [16, D] in halves
                    for half in range(2):
                        h0 = half * CH
                        OCh = ps2.tile([H, CH], F32, tag="OCh")
                        for kp in range(0, KT, 2):
                            nc.tensor.matmul(
                                OCh[:], GnT8[:, kp:kp + 2, :],
                                wv_s[:, kp:kp + 2, h0:h0 + CH],
                                start=(kp == 0), stop=(kp == KT - 2),
                                perf_mode=DRM)
                        OCsb = psb.tile([H, CH], F32, tag="OCsb")
                        nc.vector.tensor_copy(OCsb[:], OCh[:])
                        OCT = ps2.tile([128, 4 * H], F32, tag="OCT")
                        for kk in range(4):
                            k = half * 4 + kk
                            nc.tensor.transpose(
                                OCT[:, kk * H:(kk + 1) * H],
                                OCsb[:, kk * 128:(kk + 1) * 128],
                                idn_s[0:16, 0:16])
                            nc.vector.tensor_copy(
                                ocv8[0:64, k:k + 1, 0:1],
                                OCT[0:64, kk * H + 2 * k:kk * H + 2 * k + 1])
                            nc.vector.tensor_copy(
                                ocv8[64:128, k:k + 1, 0:1],
                                OCT[64:128,
                                    kk * H + 2 * k + 1:kk * H + 2 * k + 2])

                    # h_c_new natural row: ocv stationary, Wo moves
                    hcnN = psb.tile([1, D], F32, tag="hcnN")
                    for half in range(2):
                        h0 = half * CH
                        HCh = ps2.tile([1, CH], F32, tag="HCh")
                        for kp in range(0, KT, 2):
                            nc.tensor.matmul(
                                HCh[:], ocv8[:, kp:kp + 2, 0:1],
                                wo_s[:, kp:kp + 2, h0:h0 + CH],
                                start=(kp == 0), stop=(kp == KT - 2),
                                perf_mode=DRM)
                        nc.vector.scalar_tensor_tensor(
                            hcnN[:, h0:h0 + CH], HCh[:], RES / (SCL * SCL),
                            hcv_s[:, h0:h0 + CH], op0=OP.mult, op1=OP.add)
                    nc.sync.dma_start(out=outC[:], in_=hcnN[:])
                    # reshape [1, D] -> [128, KT] via PE transposes, cast fp8
                    hcT = ps2.tile([128, KT], F32, tag="vecT")
                    for k in range(KT):
                        nc.tensor.transpose(
                            hcT[:, k:k + 1],
                            hcnN[:, k * 128:(k + 1) * 128],
                            idn_s[0:1, 0:1])
                    nc.vector.tensor_copy(hcn8[:, :, 0:1], hcT[:])

                    # a0/g0 natural rows: hcn stationary, W1b/Wgb move
                    a0N = psb.tile([1, D], F32, tag="a0N")
                    g0N = psb.tile([1, D], F32, tag="g0N")
                    for dst, wsb, bias in ((a0N, w1b_s, b1v_s),
                                           (g0N, wgb_s, bgv_s)):
                        for half in range(2):
                            h0 = half * CH
                            A0h = ps2.tile([1, CH], F32, tag="A0h")
                            for kp in range(0, KT, 2):
                                nc.tensor.matmul(
                                    A0h[:], hcn8[:, kp:kp + 2, 0:1],
                                    wsb[:, kp:kp + 2, h0:h0 + CH],
                                    start=(kp == 0), stop=(kp == KT - 2),
                                    perf_mode=DRM)
                            nc.vector.scalar_tensor_tensor(
                                dst[:, h0:h0 + CH], A0h[:], 1.0 / SCL,
                                bias[:, h0:h0 + CH], op0=OP.mult, op1=OP.add)
                    for src, dst in ((a0N, a0_s), (g0N, g0_s)):
                        vT = ps2.tile([128, KT], F32, tag="vecT")
                        for k in range(KT):
                            nc.tensor.transpose(
                                vT[:, k:k + 1],
                                src[:, k * 128:(k + 1) * 128],
                                idn_s[0:1, 0:1])
                        nc.vector.tensor_copy(dst[:], vT[:])

            if variant in ("full", "nocc"):
                # =========================== PASS 2 ===========================
                with (
                    tc.tile_pool(name="p2sb", bufs=2) as sb3,
                    tc.tile_pool(name="p2st", bufs=3) as sb4,
                    tc.tile_pool(name="p2ps", bufs=2, space="PSUM") as ps3,
                ):
                    for c in range(NCH):
                        c0 = c * CH
                        hTrc = sb3.tile([128, KT, CH], F32, tag="hTrc")
                        nc.sync.dma_start(
                            out=hTrc[:],
                            in_=hT[:, c0:c0 + CH].rearrange(
                                "(k p) j -> p k j", p=128))
                        B8 = sb3.tile([128, KT, CH], F8, tag="B8")
                        for m in range(KT):
                            A = ps3.tile([128, CH], F32, tag="A")
                            for kp in range(0, KT, 2):
                                nc.tensor.matmul(
                                    A[:],
                                    w1t_s[:, kp:kp + 2, m * 128:(m + 1) * 128],
                                    h8_s[:, kp:kp + 2, c0:c0 + CH],
                                    start=(kp == 0), stop=(kp == KT - 2),
                                    perf_mode=DRM)
                            # silu = z * sigmoid(z); SILU activations force an
                            # ACT table reload (1.28 us) on every call, so
                            # build it from SIGMOID (table-resident) + DVE
                            Az = sb4.tile([128, CH], F32, tag="Az")
                            nc.vector.tensor_scalar(
                                Az[:], A[:], 1.0 / SCL, a0_s[:, m:m + 1],
                                op0=OP.mult, op1=OP.add)
                            sg = sb4.tile([128, CH], F32, tag="sg")
                            nc.scalar.activation(sg[:], Az[:], AF.Sigmoid)
                            nc.vector.tensor_mul(B8[:, m:m + 1, :], Az[:],
                                                 sg[:])
                        for m in range(KT):
                            Gt = ps3.tile([128, CH], F32, tag="Gt")
                            for kp in range(0, KT, 2):
                                nc.tensor.matmul(
                                    Gt[:],
                                    wgt_s[:, kp:kp + 2, m * 128:(m + 1) * 128],
                                    h8_s[:, kp:kp + 2, c0:c0 + CH],
                                    start=(kp == 0), stop=(kp == KT - 2),
                                    perf_mode=DRM)
                            gs = sb4.tile([128, CH], F32, tag="gs")
                            nc.scalar.activation(gs[:], Gt[:], AF.Sigmoid,
                                                 bias=g0_s[:, m:m + 1],
                                                 scale=1.0 / SCL)
                            Cp = ps3.tile([128, CH], F32, tag="Cp")
                            for kp in range(0, KT, 2):
                                nc.tensor.matmul(
                                    Cp[:],
                                    w2h_s[:, kp:kp + 2, m * 128:(m + 1) * 128],
                                    B8[:, kp:kp + 2, :],
                                    start=(kp == 0), stop=(kp == KT - 2),
                                    perf_mode=DRM)
                            t6 = sb4.tile([128, CH], F32, tag="t6")
                            nc.vector.scalar_tensor_tensor(
                                t6[:], Cp[:], b2v_s[:, m:m + 1], gs[:],
                                op0=OP.add, op1=OP.mult)
                            nc.vector.scalar_tensor_tensor(
                                hTrc[:, m:m + 1, :], t6[:], 1.0 / SCL,
                                hTrc[:, m:m + 1, :],
                                op0=OP.mult, op1=OP.add)
                        nc.sync.dma_start(
                            out=outT[:, c0:c0 + CH].rearrange(
                                "(k p) j -> p k j", p=128),
                            in_=hTrc[:])
            wres_cm.__exit__(None, None, None)
    nc.compile()
    return nc


def _get_nc():
    if "nc" not in _CACHE:
        _CACHE["nc"] = _build()
    return _CACHE["nc"]


def kernel(h, center_idx, rbf_ic, seqsep_ic, nbr_idx, local_bias,
           gamma_c, beta_c, gamma_a, beta_a,
           Wq, Wk, Wv, Wo, Wb, W1, b1, W2, b2, Wg, bg):
    global LAST_RESULTS
    f = np.float32
    f8 = ml_dtypes.float8_e4m3
    bf = ml_dtypes.bfloat16
    h = np.asarray(h, f)
    c = int(center_idx)
    rbf_ic = np.asarray(rbf_ic, f)
    seqsep_ic = np.asarray(seqsep_ic, f)
    nbr_idx = np.asarray(nbr_idx)
    local_bias = np.asarray(local_bias, f)
    gamma_c = np.asarray(gamma_c, np.float64)
    beta_c = np.asarray(beta_c, np.float64)
    gamma_a = np.asarray(gamma_a, np.float64)
    beta_a = np.asarray(beta_a, np.float64)
    Wq = np.asarray(Wq, f); Wk = np.asarray(Wk, f); Wv = np.asarray(Wv, f)
    Wo = np.asarray(Wo, f); Wb = np.asarray(Wb, f)
    W1 = np.asarray(W1, f); b1 = np.asarray(b1, f)
    W2 = np.asarray(W2, f); b2 = np.asarray(b2, f)
    Wg = np.asarray(Wg, f); bg = np.asarray(bg, f)

    # ---- host algebra (tiny, no big matmuls) ----
    hc = h[c].astype(np.float64)
    hcl = (hc - hc.mean()) / np.sqrt(hc.var() + EPS) * gamma_c + beta_c
    q = (hcl @ Wq.astype(np.float64)).reshape(H, HD)
    Qm = np.zeros((D, H), np.float64)
    for hh in range(H):
        Qm[hh * HD:(hh + 1) * HD, hh] = q[hh] / np.sqrt(HD)
    Wk1 = Wk.astype(np.float64) @ Qm                    # (D, 16)
    Wkp = (Wk1 * gamma_a[:, None]).astype(f)
    ncg = (-(Wk1 * gamma_a[:, None]).sum(0)).astype(f).reshape(H, 1)
    cbv = (Wk1 * beta_a[:, None]).sum(0).astype(f).reshape(H, 1)

    Wkp8x = np.zeros((D, 64), f)
    Wkp8x[:, :H] = SCL * Wkp
    ones16x = np.zeros((128, KT, 16 * NCH), f)
    for cc in range(NCH):
        ones16x[:, :, cc * 16 + cc] = 1.0
    ones16x = ones16x.reshape(128, KT * 16 * NCH)

    full_bias = np.zeros((N, local_bias.shape[1]), f)
    full_bias[nbr_idx] = local_bias
    bias_featT = np.ascontiguousarray(
        np.concatenate([rbf_ic, seqsep_ic, full_bias], axis=1).T)  # (128, N)

    hT_full = np.ascontiguousarray(h.T)                 # (D, N)
    h8_full = h.astype(f8)                              # (N, D) fp8
    h8T_full = np.ascontiguousarray(h8_full.T)          # (D, N) fp8

    gamma_a32 = gamma_a.astype(f)
    beta_a32 = beta_a.astype(f)
    shared = {
        "Wkp8": Wkp8x.astype(f8), "Wb": (SCL * Wb).astype(bf),
        "W1t8": (SCL * np.ascontiguousarray(W1[:D])).astype(f8),
        "Wgt8": (SCL * np.ascontiguousarray(Wg[:D])).astype(f8),
        "W2h8": (SCL * RES * W2).astype(f8),
        "Wv8": (SCL * Wv).astype(f8), "Wo8": (SCL * Wo).astype(f8),
        "W1b8": (SCL * np.ascontiguousarray(W1[D:])).astype(f8),
        "Wgb8": (SCL * np.ascontiguousarray(Wg[D:])).astype(f8),
        "idn": np.eye(128, dtype=f),
        "ones16": ones16x.astype(f8),
        "ncg1": (SCL * ncg).reshape(1, H).astype(bf),
        "eps16": np.full((16, 1), EPS, f),
        "cbv": cbv,
        "gb16": np.ascontiguousarray(np.broadcast_to(gamma_a32, (H, D))),
        "bb16": np.ascontiguousarray(np.broadcast_to(beta_a32, (H, D))),
        "hcvN": h[c:c + 1].copy(),
        "b1N": b1.reshape(1, D).copy(),
        "bgN": bg.reshape(1, D).copy(),
        "b2v": np.ascontiguousarray((SCL * RES * b2).reshape(KT, 128).T),
    }
    in_maps = []
    for i in range(NCORES):
        r0 = i * NS
        m = dict(shared)
        m["hT"] = np.ascontiguousarray(hT_full[:, r0:r0 + NS])
        m["h8T"] = np.ascontiguousarray(h8T_full[:, r0:r0 + NS])
        m["hN8"] = h8_full[r0:r0 + NS]
        m["bT"] = np.ascontiguousarray(bias_featT[:, r0:r0 + NS]).astype(bf)
        in_maps.append(m)

    nc = _get_nc()
    trace = bool(int(os.environ.get("KERNEL_TRACE", "0")))
    res = run_bass_kernel_spmd(nc, in_maps, core_ids=list(range(NCORES)),
                               trace=trace)
    LAST_RESULTS = res

    out = np.empty((N, D), f)
    for i in range(NCORES):
        out[i * NS:(i + 1) * NS] = res.results[i]["outT"].T
    out[c] = res.results[0]["outC"].reshape(D)
    return out


# revision 41
# speedup vs baseline: 1.0874x; 1.0874x over previous
"""Trainium2 Bass kernel for CenterGeoAttention (N=65536, D=1024, H=16).

Strategy (row-shard N across 8 cores, fp8 DoubleRow compute):

  - All heavy matmuls run in fp8e4m3 with MatmulPerfMode.DoubleRow
    (K=256 per instruction, 2 multiplies/cycle) against an SBUF-resident
    fp8 copy of the h shard (h8_s, 8 MiB).  Weights are pre-scaled by 64
    on the host so their N(0, 0.02) entries sit in fp8's normal range;
    the 1/64 descale folds into activation-scale / scalar_tensor_tensor.
  - LayerNorm is folded into rank-1 corrections (as before): the
    logits sweep's stationary operand carries [64*Wkp | ones] so row 16
    of the output is the per-row sum (mean) for free; the sumsq sweep
    uses fp8 squares computed on the Scalar engine.
  - The weighted V sum never materializes V: G = (p*r)^T @ h8 via fp8
    DoubleRow, AllReduce-add [G | PRM | S], then the tiny post-AR GEMV
    chain (out_center, h_c_new, a0, g0) runs on fp8 weights.
  - Pass 2: 3 big DR matmuls per chunk (h@W1t, h@Wgt, silu@W2) read the
    resident h8; fp32 h streams in only for the residual add.
  - Wv/Wo/W1b/Wgb prefetch during pass-1 chunks; W1t/Wgt/W2h load
    during the AllReduce so the inter-pass valley is just AR latency.
"""

import os
import ml_dtypes
import numpy as np

import concourse.bass as bass
import concourse.bacc as bacc
import concourse.tile as tile
import concourse.mybir as mybir
from concourse.bass_utils import run_bass_kernel_spmd

F32 = mybir.dt.float32
F8 = mybir.dt.float8e4
BF16 = mybir.dt.bfloat16
AF = mybir.ActivationFunctionType
OP = mybir.AluOpType
AX = mybir.AxisListType
DRM = mybir.MatmulPerfMode.DoubleRow

NCORES = 8
N, D, H, HD, BIAS = 65536, 1024, 16, 64, 128
NS = N // NCORES            # 8192 rows per core
CH = 512                    # row-chunk
NCH = NS // CH              # 16 chunks
KT = D // 128               # 8 feature tiles
EPS = 1e-5
RES = 0.5
SCL = 64.0                  # fp8 weight pre-scale

_CACHE = {}
LAST_RESULTS = None  # BassKernelResults from the most recent run (for test.py)


def _build(ncores=NCORES, variant="full", nch=NCH):
    nc = bacc.Bacc("TRN2", target_bir_lowering=False, debug=False,
                   num_devices=ncores)

    def din(name, shape, dt=F32):
        return nc.dram_tensor(name, list(shape), dt, kind="ExternalInput").ap()

    # per-core tensors
    hT = din("hT", (D, NS))               # h_shard^T fp32 (residual stream)
    h8T = din("h8T", (D, NS), F8)         # h_shard^T fp8
    hN8 = din("hN8", (NS, D), F8)         # h_shard natural fp8
    bT = din("bT", (BIAS, NS), BF16)      # bias_feat^T shard
    # shared weights
    Wkp8 = din("Wkp8", (D, 64), F8)       # [64*Wkp | pad | ones@32 | pad]
    Wb = din("Wb", (BIAS, H), BF16)       # 64*Wb
    W1t8 = din("W1t8", (D, D), F8)        # 64*W1[:D]
    Wgt8 = din("Wgt8", (D, D), F8)        # 64*Wg[:D]
    W2h8 = din("W2h8", (D, D), F8)        # 64*RES*W2
    Wv8 = din("Wv8", (D, D), F8)          # 64*Wv
    Wo8 = din("Wo8", (D, D), F8)          # 64*Wo
    W1b8 = din("W1b8", (D, D), F8)        # 64*W1[D:]
    Wgb8 = din("Wgb8", (D, D), F8)        # 64*Wg[D:]
    # small constants
    idn = din("idn", (128, 128), F32)
    ones16 = din("ones16", (128, KT * 16 * NCH), F8)  # block c = ones in col c
    ncg1 = din("ncg1", (1, H), BF16)      # 64*(-cg) as K=1 stationary
    eps16 = din("eps16", (16, 1), F32)
    cbv = din("cbv", (H, 1), F32)         # cb per head (exp bias)
    gb16 = din("gb16", (H, D), F32)       # gamma_a broadcast rows
    bb16 = din("bb16", (H, D), F32)       # beta_a broadcast rows
    hcvN = din("hcvN", (1, D), F32)       # h[c] natural row
    b1N = din("b1N", (1, D), F32)
    bgN = din("bgN", (1, D), F32)
    b2v = din("b2v", (128, KT), F32)      # 64*RES*b2

    outT = nc.dram_tensor("outT", [D, NS], F32, kind="ExternalOutput").ap()
    outC = nc.dram_tensor("outC", [1, D], F32, kind="ExternalOutput").ap()

    with tile.TileContext(nc) as tc:
        with (
            tc.tile_pool(name="persist", bufs=1) as pp,
            tc.tile_pool(name="dram", bufs=1, space="DRAM") as dram,
        ):
            # ---- long-lived small tiles ----
            idn_s = pp.tile([128, 128], F32, tag="idn")
            nc.sync.dma_start(out=idn_s[:], in_=idn[:])
            cbv_s = pp.tile([H, 1], F32, tag="cbv")
            nc.sync.dma_start(out=cbv_s[:], in_=cbv[:])
            b2v_s = pp.tile([128, KT], F32, tag="b2v")
            nc.sync.dma_start(out=b2v_s[:], in_=b2v[:])
            ones16_s = pp.tile([128, KT, 16 * NCH], F8, tag="ones16")
            nc.sync.dma_start(out=ones16_s[:], in_=ones16[:])
            ncg1_s = pp.tile([1, H], BF16, tag="ncg1")
            nc.sync.dma_start(out=ncg1_s[:], in_=ncg1[:])
            eps16_s = pp.tile([16, 1], F32, tag="eps16")
            nc.sync.dma_start(out=eps16_s[:], in_=eps16[:])
            Wkp8_s = pp.tile([128, KT, 64], F8, tag="Wkp8")
            for k in range(KT):
                nc.sync.dma_start(out=Wkp8_s[:, k:k + 1, :],
                                  in_=Wkp8[k * 128:(k + 1) * 128, :])
            Wb_s = pp.tile([BIAS, H], BF16, tag="Wb")
            nc.sync.dma_start(out=Wb_s[:], in_=Wb[:])

            h8_s = pp.tile([128, KT, NS], F8, tag="h8")
            Gacc = pp.tile([H, D], F32, tag="Gacc")
            sCols = pp.tile([H, NCH], F32, tag="sCols")
            g0_s = pp.tile([128, KT], F32, tag="g0")
            a0_s = pp.tile([128, KT], F32, tag="a0")
            GnT8 = pp.tile([128, KT, H], F8, tag="GnT8")
            ocv8 = pp.tile([128, KT, 16], F8, tag="ocv8")
            hcn8 = pp.tile([128, KT, 16], F8, tag="hcn8")

            # resident fp8 weights, loaded during pass 1 / the AR valley
            wres_cm = tc.tile_pool(name="wres", bufs=1)
            wres = wres_cm.__enter__()
            wv_s = wres.tile([128, KT, D], F8, tag="wv")
            wo_s = wres.tile([128, KT, D], F8, tag="wo")
            w1b_s = wres.tile([128, KT, D], F8, tag="w1b")
            wgb_s = wres.tile([128, KT, D], F8, tag="wgb")
            w1t_s = wres.tile([128, KT, D], F8, tag="w1t")
            wgt_s = wres.tile([128, KT, D], F8, tag="wgt")
            w2h_s = wres.tile([128, KT, D], F8, tag="w2h")
            PREFETCH = [(wv_s, Wv8), (wo_s, Wo8), (w1b_s, W1b8), (wgb_s, Wgb8)]
            VALLEY = [(w1t_s, W1t8), (wgt_s, Wgt8), (w2h_s, W2h8)]

            # =========================== PASS 1 ===========================
            # -- loop A: per-row sum and sum-of-squares, banked into a
            #    [16, CH] psum tile (chunk c -> row c via one-hot stationary)
            strips_cm = tc.tile_pool(name="strips", bufs=1)
            strips = strips_cm.__enter__()
            rb_all = strips.tile([1, NS], F32, tag="rb_all")  # 1/sd strip
            tm_all = strips.tile([1, NS], BF16, tag="tm_all")  # mean strip
            with (
                tc.tile_pool(name="pAsb", bufs=2) as sbA,
                tc.tile_pool(name="pAps", bufs=1, space="PSUM") as psS,
                tc.tile_pool(name="pAsb1", bufs=1) as sbM,
            ):
                SM16 = psS.tile([16, CH], F32, tag="SM16")
                SQ16 = psS.tile([16, CH], F32, tag="SQ16")
                for c in range(nch):
                    c0 = c * CH
                    nc.sync.dma_start(
                        out=h8_s[:, :, c0:c0 + CH],
                        in_=h8T[:, c0:c0 + CH].rearrange(
                            "(k p) j -> p k j", p=128))
                    if 1 <= c <= len(PREFETCH):
                        wsb, wd = PREFETCH[c - 1]
                        nc.scalar.dma_start(
                            out=wsb[:],
                            in_=wd[:].rearrange("(k p) j -> p k j", p=128))
                    sq8 = sbA.tile([128, KT, CH], F8, tag="sq8")
                    nc.vector.tensor_mul(sq8[:, 0:4, :],
                                         h8_s[:, 0:4, c0:c0 + CH],
                                         h8_s[:, 0:4, c0:c0 + CH])
                    nc.vector.tensor_mul(sq8[:, 4:8, :],
                                         h8_s[:, 4:8, c0:c0 + CH],
                                         h8_s[:, 4:8, c0:c0 + CH])
                    oc0 = c * 16
                    for kp in range(0, KT, 2):
                        nc.tensor.matmul(SM16[:],
                                         ones16_s[:, kp:kp + 2, oc0:oc0 + 16],
                                         h8_s[:, kp:kp + 2, c0:c0 + CH],
                                         start=(c == 0 and kp == 0),
                                         stop=(c == nch - 1 and kp == KT - 2),
                                         perf_mode=DRM)
                    for kp in range(0, KT, 2):
                        nc.tensor.matmul(SQ16[:],
                                         ones16_s[:, kp:kp + 2, oc0:oc0 + 16],
                                         sq8[:, kp:kp + 2, :],
                                         start=(c == 0 and kp == 0),
                                         stop=(c == nch - 1 and kp == KT - 2),
                                         perf_mode=DRM)
                # -- mid: batched LayerNorm stats for all 16 chunks at once
                tm16 = sbM.tile([16, CH], F32, tag="tm16")
                nc.vector.tensor_scalar_mul(tm16[:], SM16[:], 1.0 / D)
                msq16 = sbM.tile([16, CH], F32, tag="msq16")
                nc.vector.tensor_mul(msq16[:], tm16[:], tm16[:])
                var16 = sbM.tile([16, CH], F32, tag="var16")
                nc.vector.scalar_tensor_tensor(
                    var16[:], SQ16[:], 1.0 / D, msq16[:],
                    op0=OP.mult, op1=OP.subtract)
                sd16 = sbM.tile([16, CH], F32, tag="sd16")
                nc.scalar.activation(sd16[:], var16[:], AF.Sqrt,
                                     bias=eps16_s[:, 0:1])
                r16 = sbM.tile([16, CH], F32, tag="r16")
                nc.vector.reciprocal_approx_fast(r16[:], sd16[:])
                tmb16 = sbM.tile([16, CH], BF16, tag="tmb16")
                nc.vector.tensor_copy(tmb16[:], tm16[:])
                # reshape [16, CH] -> [1, NS] strips (row c -> cols c*CH...)
                nc.sync.dma_start(out=rb_all[:], in_=r16[:])
                nc.sync.dma_start(out=tm_all[:], in_=tmb16[:])

            # -- loop B: logits, softmax, and the G accumulation
            psG_cm = tc.tile_pool(name="psG", bufs=1, space="PSUM")
            psG = psG_cm.__enter__()
            G = psG.tile([H, D], F32, tag="G")
            with (
                tc.tile_pool(name="p1sb", bufs=1) as sb1,
                tc.tile_pool(name="p1sb2", bufs=2) as sb2,
                tc.tile_pool(name="p1psA", bufs=2, space="PSUM") as psA,
                tc.tile_pool(name="p1psB", bufs=1, space="PSUM") as psB,
            ):
                for c in range(nch):
                    c0 = c * CH
                    # pass-2 weights stream early so they are on-chip well
                    # before the AllReduce (concurrent bulk DMA slows it)
                    if c in (5, 9, 13):
                        wsb, wd = VALLEY[(c - 5) // 4]
                        nc.scalar.dma_start(
                            out=wsb[:],
                            in_=wd[:].rearrange("(k p) j -> p k j", p=128))
                    hN8c = sb2.tile([128, 4, D], F8, tag="hN8c")
                    nc.sync.dma_start(
                        out=hN8c[:],
                        in_=hN8[c0:c0 + CH, :].rearrange(
                            "(jj p) d -> p jj d", p=128))
                    bTc = sb2.tile([BIAS, CH], BF16, tag="bTc")
                    nc.sync.dma_start(out=bTc[:], in_=bT[:, c0:c0 + CH])

                    # Lp = 64*(Wkp^T h8 + ncg x m)  (ncg term via K=1 matmul)
                    Lp = psA.tile([H, CH], F32, tag="Lp")
                    for kp in range(0, KT, 2):
                        nc.tensor.matmul(Lp[:], Wkp8_s[:, kp:kp + 2, 0:16],
                                         h8_s[:, kp:kp + 2, c0:c0 + CH],
                                         start=(kp == 0), stop=False,
                                         perf_mode=DRM)
                    nc.tensor.matmul(Lp[:], ncg1_s[:],
                                     tm_all[:, c0:c0 + CH],
                                     start=False, stop=True)
                    L2 = psB.tile([H, CH], F32, tag="L2")
                    nc.tensor.matmul(L2[:], Wb_s[:], bTc[:],
                                     start=True, stop=True)

                    rb16 = sb2.tile([H, CH], F32, tag="rb16")
                    nc.gpsimd.partition_broadcast(rb16[:],
                                                  rb_all[:, c0:c0 + CH])
                    t3 = sb1.tile([H, CH], F32, tag="t3")
                    nc.vector.tensor_mul(t3[:], Lp[:], rb16[:])
                    t5 = sb2.tile([H, CH], F32, tag="t5")
                    nc.vector.tensor_add(t5[:], t3[:], L2[:])
                    pT = sb2.tile([H, CH], F32, tag="pT")
                    nc.scalar.activation(pT[:], t5[:], AF.Exp,
                                         bias=cbv_s[:, 0:1], scale=1.0 / SCL,
                                         accum_out=sCols[:, c:c + 1])
                    prT = sb2.tile([H, CH], F32, tag="prT")
                    nc.vector.tensor_mul(prT[:], pT[:], rb16[:])
                    # transpose p*r to natural fp8 and accumulate G
                    tp = psB.tile([128, 4 * H], F32, tag="tp")
                    for j in range(4):
                        nc.tensor.transpose(
                            tp[:, j * H:(j + 1) * H],
                            prT[:, j * 128:(j + 1) * 128],
                            idn_s[0:16, 0:16])
                    pr8 = sb2.tile([128, 4, H], F8, tag="pr8")
                    nc.vector.tensor_copy(pr8[:], tp[:])
                    for jp in (0, 2):
                        for half in range(2):
                            h0 = half * CH
                            nc.tensor.matmul(
                                G[:, h0:h0 + CH],
                                pr8[:, jp:jp + 2, :],
                                hN8c[:, jp:jp + 2, h0:h0 + CH],
                                start=(c == 0 and jp == 0),
                                stop=(c == nch - 1 and jp == 2),
                                perf_mode=DRM)
                nc.vector.tensor_copy(Gacc[:], G[:])
                if variant == "p1":
                    nc.sync.dma_start(out=outT[0:H, 0:D], in_=Gacc[:])
                    nc.sync.dma_start(out=outT[H:2 * H, 0:NCH], in_=sCols[:])
            strips_cm.__exit__(None, None, None)

            if variant != "p1":
                psG_cm.__exit__(None, None, None)
                # ---- local partials -> AllReduce ----
                # PRM = row-sum(G)/D exactly (sum_d G[h,d] = D * sum p*r*m)
                S16 = pp.tile([H, 1], F32, tag="S16")
                nc.vector.reduce_sum(S16[:], sCols[:], axis=AX.X)
                PRM16 = pp.tile([H, 1], F32, tag="PRM16")
                nc.vector.reduce_sum(PRM16[:], Gacc[:], axis=AX.X)
                nc.vector.tensor_scalar_mul(PRM16[:], PRM16[:], 1.0 / D)

                arin = dram.tile([H, D + 2], F32, tag="arin")
                arout = dram.tile([H, D + 2], F32, tag="arout")
                nc.sync.dma_start(out=arin[:, 0:D], in_=Gacc[:])
                nc.sync.dma_start(out=arin[:, D:D + 1], in_=PRM16[:])
                nc.sync.dma_start(out=arin[:, D + 1:D + 2], in_=S16[:])
                if variant == "nocc":
                    nc.sync.dma_start(out=arout[:], in_=arin[:])
                else:
                    nc.gpsimd.collective_compute(
                        "AllReduce", OP.add,
                        replica_groups=[list(range(ncores))],
                        ins=[arin.opt()], outs=[arout.opt()])
                # ---- G corrections + normalize + tiny GEMV chain ----
                with (
                    tc.tile_pool(name="postsb", bufs=1) as psb,
                    tc.tile_pool(name="postps", bufs=1, space="PSUM") as ps2,
                ):
                    hcv_s = psb.tile([1, D], F32, tag="hcv")
                    nc.sync.dma_start(out=hcv_s[:], in_=hcvN[:])
                    b1v_s = psb.tile([1, D], F32, tag="b1v")
                    nc.sync.dma_start(out=b1v_s[:], in_=b1N[:])
                    bgv_s = psb.tile([1, D], F32, tag="bgv")
                    nc.sync.dma_start(out=bgv_s[:], in_=bgN[:])
                    gb16_s = psb.tile([H, D], F32, tag="gb16")
                    nc.sync.dma_start(out=gb16_s[:], in_=gb16[:])
                    bb16_s = psb.tile([H, D], F32, tag="bb16")
                    nc.sync.dma_start(out=bb16_s[:], in_=bb16[:])
                    Gar = psb.tile([H, D], F32, tag="Gar")
                    nc.sync.dma_start(out=Gar[:], in_=arout[:, 0:D])
                    PSar = psb.tile([H, 2], F32, tag="PSar")
                    nc.sync.dma_start(out=PSar[:], in_=arout[:, D:D + 2])
                    Gn = psb.tile([H, D], F32, tag="Gn")
                    nc.vector.tensor_scalar_sub(Gn[:], Gar[:], PSar[:, 0:1])
                    nc.vector.tensor_mul(Gn[:], Gn[:], gb16_s[:])
                    nc.vector.scalar_tensor_tensor(
                        Gn[:], bb16_s[:], PSar[:, 1:2], Gn[:],
                        op0=OP.mult, op1=OP.add)
                    sr = psb.tile([H, 1], F32, tag="sr")
                    nc.vector.reciprocal(sr[:], PSar[:, 1:2])
                    nc.vector.tensor_scalar_mul(Gn[:], Gn[:], sr[:, 0:1])

                    tpg = ps2.tile([128, KT * H], F32, tag="tpg")
                    for m in range(KT):
                        nc.tensor.transpose(
                            tpg[:, m * H:(m + 1) * H],
                            Gn[:, m * 128:(m + 1) * 128],
                            idn_s[0:16, 0:16])
                    nc.vector.tensor_copy(GnT8[:], tpg[:])

                    # out_center natural: Gn row h dot Wv columns (weights
                    # move, Gn^T stationary) -> [16, D] in halves
                    for half in range(2):
                        h0 = half * CH
                        OCh = ps2.tile([H, CH], F32, tag="OCh")
                        for kp in range(0, KT, 2):
                            nc.tensor.matmul(
                                OCh[:], GnT8[:, kp:kp + 2, :],
                                wv_s[:, kp:kp + 2, h0:h0 + CH],
                                start=(kp == 0), stop=(kp == KT - 2),
                                perf_mode=DRM)
                        OCsb = psb.tile([H, CH], F32, tag="OCsb")
                        nc.vector.tensor_copy(OCsb[:], OCh[:])
                        OCT = ps2.tile([128, 4 * H], F32, tag="OCT")
                        for kk in range(4):
                            k = half * 4 + kk
                            nc.tensor.transpose(
                                OCT[:, kk * H:(kk + 1) * H],
                                OCsb[:, kk * 128:(kk + 1) * 128],
                                idn_s[0:16, 0:16])
                            nc.vector.tensor_copy(
                                ocv8[0:64, k:k + 1, 0:1],
                                OCT[0:64, kk * H + 2 * k:kk * H + 2 * k + 1])
                            nc.vector.tensor_copy(
                                ocv8[64:128, k:k + 1, 0:1],
                                OCT[64:128,
                                    kk * H + 2 * k + 1:kk * H + 2 * k + 2])

                    # h_c_new natural row: ocv stationary, Wo moves
                    hcnN = psb.tile([1, D], F32, tag="hcnN")
                    for half in range(2):
                        h0 = half * CH
                        HCh = ps2.tile([1, CH], F32, tag="HCh")
                        for kp in range(0, KT, 2):
                            nc.tensor.matmul(
                                HCh[:], ocv8[:, kp:kp + 2, 0:1],
                                wo_s[:, kp:kp + 2, h0:h0 + CH],
                                start=(kp == 0), stop=(kp == KT - 2),
                                perf_mode=DRM)
                        nc.vector.scalar_tensor_tensor(
                            hcnN[:, h0:h0 + CH], HCh[:], RES / (SCL * SCL),
                            hcv_s[:, h0:h0 + CH], op0=OP.mult, op1=OP.add)
                    nc.sync.dma_start(out=outC[:], in_=hcnN[:])
                    # reshape [1, D] -> [128, KT] via PE transposes, cast fp8
                    hcT = ps2.tile([128, KT], F32, tag="vecT")
                    for k in range(KT):
                        nc.tensor.transpose(
                            hcT[:, k:k + 1],
                            hcnN[:, k * 128:(k + 1) * 128],
                            idn_s[0:1, 0:1])
                    nc.vector.tensor_copy(hcn8[:, :, 0:1], hcT[:])

                    # a0/g0 natural rows: hcn stationary, W1b/Wgb move
                    a0N = psb.tile([1, D], F32, tag="a0N")
                    g0N = psb.tile([1, D], F32, tag="g0N")
                    for dst, wsb, bias in ((a0N, w1b_s, b1v_s),
                                           (g0N, wgb_s, bgv_s)):
                        for half in range(2):
                            h0 = half * CH
                            A0h = ps2.tile([1, CH], F32, tag="A0h")
                            for kp in range(0, KT, 2):
                                nc.tensor.matmul(
                                    A0h[:], hcn8[:, kp:kp + 2, 0:1],
                                    wsb[:, kp:kp + 2, h0:h0 + CH],
                                    start=(kp == 0), stop=(kp == KT - 2),
                                    perf_mode=DRM)
                            nc.vector.scalar_tensor_tensor(
                                dst[:, h0:h0 + CH], A0h[:], 1.0 / SCL,
                                bias[:, h0:h0 + CH], op0=OP.mult, op1=OP.add)
                    for src, dst in ((a0N, a0_s), (g0N, g0_s)):
                        vT = ps2.tile([128, KT], F32, tag="vecT")
                        for k in range(KT):
                            nc.tensor.transpose(
                                vT[:, k:k + 1],
                                src[:, k * 128:(k + 1) * 128],
                                idn_s[0:1, 0:1])
                        nc.vector.tensor_copy(dst[:], vT[:])

            if variant in ("full", "nocc"):
                # =========================== PASS 2 ===========================
                with (
                    tc.tile_pool(name="p2sb", bufs=2) as sb3,
                    tc.tile_pool(name="p2st", bufs=3) as sb4,
                    tc.tile_pool(name="p2ps", bufs=2, space="PSUM") as ps3,
                ):
                    for c in range(NCH):
                        c0 = c * CH
                        hTrc = sb3.tile([128, KT, CH], F32, tag="hTrc")
                        nc.sync.dma_start(
                            out=hTrc[:],
                            in_=hT[:, c0:c0 + CH].rearrange(
                                "(k p) j -> p k j", p=128))
                        B8 = sb3.tile([128, KT, CH], F8, tag="B8")
                        for m in range(KT):
                            A = ps3.tile([128, CH], F32, tag="A")
                            for kp in range(0, KT, 2):
                                nc.tensor.matmul(
                                    A[:],
                                    w1t_s[:, kp:kp + 2, m * 128:(m + 1) * 128],
                                    h8_s[:, kp:kp + 2, c0:c0 + CH],
                                    start=(kp == 0), stop=(kp == KT - 2),
                                    perf_mode=DRM)
                            # silu = z * sigmoid(z); SILU activations force an
                            # ACT table reload (1.28 us) on every call, so
                            # build it from SIGMOID (table-resident) + DVE
                            Az = sb4.tile([128, CH], F32, tag="Az")
                            nc.vector.tensor_scalar(
                                Az[:], A[:], 1.0 / SCL, a0_s[:, m:m + 1],
                                op0=OP.mult, op1=OP.add)
                            sg = sb4.tile([128, CH], F32, tag="sg")
                            nc.scalar.activation(sg[:], Az[:], AF.Sigmoid)
                            nc.vector.tensor_mul(B8[:, m:m + 1, :], Az[:],
                                                 sg[:])
                        for m in range(KT):
                            Gt = ps3.tile([128, CH], F32, tag="Gt")
                            for kp in range(0, KT, 2):
                                nc.tensor.matmul(
                                    Gt[:],
                                    wgt_s[:, kp:kp + 2, m * 128:(m + 1) * 128],
                                    h8_s[:, kp:kp + 2, c0:c0 + CH],
                                    start=(kp == 0), stop=(kp == KT - 2),
                                    perf_mode=DRM)
                            gs = sb4.tile([128, CH], F32, tag="gs")
                            nc.scalar.activation(gs[:], Gt[:], AF.Sigmoid,
                                                 bias=g0_s[:, m:m + 1],
                                                 scale=1.0 / SCL)
                            Cp = ps3.tile([128, CH], F32, tag="Cp")
                            for kp in range(0, KT, 2):
                                nc.tensor.matmul(
                                    Cp[:],
                                    w2h_s[:, kp:kp + 2, m * 128:(m + 1) * 128],
                                    B8[:, kp:kp + 2, :],
                                    start=(kp == 0), stop=(kp == KT - 2),
                                    perf_mode=DRM)
                            t6 = sb4.tile([128, CH], F32, tag="t6")
                            nc.vector.scalar_tensor_tensor(
                                t6[:], Cp[:], b2v_s[:, m:m + 1], gs[:],
                                op0=OP.add, op1=OP.mult)
                            nc.vector.scalar_tensor_tensor(
                                hTrc[:, m:m + 1, :], t6[:], 1.0 / SCL,
                                hTrc[:, m:m + 1, :],
                                op0=OP.mult, op1=OP.add)
                        nc.sync.dma_start(
                            out=outT[:, c0:c0 + CH].rearrange(
                                "(k p) j -> p k j", p=128),
                            in_=hTrc[:])
            wres_cm.__exit__(None, None, None)
    nc.compile()
    return nc


def _get_nc():
    if "nc" not in _CACHE:
        _CACHE["nc"] = _build()
    return _CACHE["nc"]


def kernel(h, center_idx, rbf_ic, seqsep_ic, nbr_idx, local_bias,
           gamma_c, beta_c, gamma_a, beta_a,
           Wq, Wk, Wv, Wo, Wb, W1, b1, W2, b2, Wg, bg):
    global LAST_RESULTS
    f = np.float32
    f8 = ml_dtypes.float8_e4m3
    bf = ml_dtypes.bfloat16
    h = np.asarray(h, f)
    c = int(center_idx)
    rbf_ic = np.asarray(rbf_ic, f)
    seqsep_ic = np.asarray(seqsep_ic, f)
    nbr_idx = np.asarray(nbr_idx)
    local_bias = np.asarray(local_bias, f)
    gamma_c = np.asarray(gamma_c, np.float64)
    beta_c = np.asarray(beta_c, np.float64)
    gamma_a = np.asarray(gamma_a, np.float64)
    beta_a = np.asarray(beta_a, np.float64)
    Wq = np.asarray(Wq, f); Wk = np.asarray(Wk, f); Wv = np.asarray(Wv, f)
    Wo = np.asarray(Wo, f); Wb = np.asarray(Wb, f)
    W1 = np.asarray(W1, f); b1 = np.asarray(b1, f)
    W2 = np.asarray(W2, f); b2 = np.asarray(b2, f)
    Wg = np.asarray(Wg, f); bg = np.asarray(bg, f)

    # ---- host algebra (tiny, no big matmuls) ----
    hc = h[c].astype(np.float64)
    hcl = (hc - hc.mean()) / np.sqrt(hc.var() + EPS) * gamma_c + beta_c
    q = (hcl @ Wq.astype(np.float64)).reshape(H, HD)
    Qm = np.zeros((D, H), np.float64)
    for hh in range(H):
        Qm[hh * HD:(hh + 1) * HD, hh] = q[hh] / np.sqrt(HD)
    Wk1 = Wk.astype(np.float64) @ Qm                    # (D, 16)
    Wkp = (Wk1 * gamma_a[:, None]).astype(f)
    ncg = (-(Wk1 * gamma_a[:, None]).sum(0)).astype(f).reshape(H, 1)
    cbv = (Wk1 * beta_a[:, None]).sum(0).astype(f).reshape(H, 1)

    Wkp8x = np.zeros((D, 64), f)
    Wkp8x[:, :H] = SCL * Wkp
    ones16x = np.zeros((128, KT, 16 * NCH), f)
    for cc in range(NCH):
        ones16x[:, :, cc * 16 + cc] = 1.0
    ones16x = ones16x.reshape(128, KT * 16 * NCH)

    full_bias = np.zeros((N, local_bias.shape[1]), f)
    full_bias[nbr_idx] = local_bias
    bias_featT = np.ascontiguousarray(
        np.concatenate([rbf_ic, seqsep_ic, full_bias], axis=1).T)  # (128, N)

    hT_full = np.ascontiguousarray(h.T)                 # (D, N)
    h8_full = h.astype(f8)                              # (N, D) fp8
    h8T_full = np.ascontiguousarray(h8_full.T)          # (D, N) fp8

    gamma_a32 = gamma_a.astype(f)
    beta_a32 = beta_a.astype(f)
    shared = {
        "Wkp8": Wkp8x.astype(f8), "Wb": (SCL * Wb).astype(bf),
        "W1t8": (SCL * np.ascontiguousarray(W1[:D])).astype(f8),
        "Wgt8": (SCL * np.ascontiguousarray(Wg[:D])).astype(f8),
        "W2h8": (SCL * RES * W2).astype(f8),
        "Wv8": (SCL * Wv).astype(f8), "Wo8": (SCL * Wo).astype(f8),
        "W1b8": (SCL * np.ascontiguousarray(W1[D:])).astype(f8),
        "Wgb8": (SCL * np.ascontiguousarray(Wg[D:])).astype(f8),
        "idn": np.eye(128, dtype=f),
        "ones16": ones16x.astype(f8),
        "ncg1": (SCL * ncg).reshape(1, H).astype(bf),
        "eps16": np.full((16, 1), EPS, f),
        "cbv": cbv,
        "gb16": np.ascontiguousarray(np.broadcast_to(gamma_a32, (H, D))),
        "bb16": np.ascontiguousarray(np.broadcast_to(beta_a32, (H, D))),
        "hcvN": h[c:c + 1].copy(),
        "b1N": b1.reshape(1, D).copy(),
        "bgN": bg.reshape(1, D).copy(),
        "b2v": np.ascontiguousarray((SCL * RES * b2).reshape(KT, 128).T),
    }
    in_maps = []
    for i in range(NCORES):
        r0 = i * NS
        m = dict(shared)
        m["hT"] = np.ascontiguousarray(hT_full[:, r0:r0 + NS])
        m["h8T"] = np.ascontiguousarray(h8T_full[:, r0:r0 + NS])
        m["hN8"] = h8_full[r0:r0 + NS]
        m["bT"] = np.ascontiguousarray(bias_featT[:, r0:r0 + NS]).astype(bf)
        in_maps.append(m)

    nc = _get_nc()
    trace = bool(int(os.environ.get("KERNEL_TRACE", "0")))
    res = run_bass_kernel_spmd(nc, in_maps, core_ids=list(range(NCORES)),
                               trace=trace)
    LAST_RESULTS = res

    out = np.empty((N, D), f)
    for i in range(NCORES):
        out[i * NS:(i + 1) * NS] = res.results[i]["outT"].T
    out[c] = res.results[0]["outC"].reshape(D)
    return out


# revision 47
# speedup vs baseline: 1.1040x; 1.0153x over previous
"""Trainium2 Bass kernel for CenterGeoAttention (N=65536, D=1024, H=16).

Strategy (row-shard N across 8 cores, fp8 DoubleRow compute):

  - All heavy matmuls run in fp8e4m3 with MatmulPerfMode.DoubleRow
    (K=256 per instruction, 2 multiplies/cycle) against an SBUF-resident
    fp8 copy of the h shard (h8_s, 8 MiB).  Weights are pre-scaled by 64
    on the host so their N(0, 0.02) entries sit in fp8's normal range;
    the 1/64 descale folds into activation-scale / scalar_tensor_tensor.
  - LayerNorm is folded into rank-1 corrections (as before): the
    logits sweep's stationary operand carries [64*Wkp | ones] so row 16
    of the output is the per-row sum (mean) for free; the sumsq sweep
    uses fp8 squares computed on the Scalar engine.
  - The weighted V sum never materializes V: G = (p*r)^T @ h8 via fp8
    DoubleRow, AllReduce-add [G | PRM | S], then the tiny post-AR GEMV
    chain (out_center, h_c_new, a0, g0) runs on fp8 weights.
  - Pass 2: 3 big DR matmuls per chunk (h@W1t, h@Wgt, silu@W2) read the
    resident h8; fp32 h streams in only for the residual add.
  - Wv/Wo/W1b/Wgb prefetch during pass-1 chunks; W1t/Wgt/W2h load
    during the AllReduce so the inter-pass valley is just AR latency.
"""

import os
import ml_dtypes
import numpy as np

import concourse.bass as bass
import concourse.bacc as bacc
import concourse.tile as tile
import concourse.mybir as mybir
from concourse.bass_utils import run_bass_kernel_spmd

F32 = mybir.dt.float32
F8 = mybir.dt.float8e4
BF16 = mybir.dt.bfloat16
AF = mybir.ActivationFunctionType
OP = mybir.AluOpType
AX = mybir.AxisListType
DRM = mybir.MatmulPerfMode.DoubleRow

NCORES = 8
N, D, H, HD, BIAS = 65536, 1024, 16, 64, 128
NS = N // NCORES            # 8192 rows per core
CH = 512                    # row-chunk
NCH = NS // CH              # 16 chunks
KT = D // 128               # 8 feature tiles
EPS = 1e-5
RES = 0.5
SCL = 64.0                  # fp8 weight pre-scale

_CACHE = {}
LAST_RESULTS = None  # BassKernelResults from the most recent run (for test.py)


def _build(ncores=NCORES, variant="full", nch=NCH):
    nc = bacc.Bacc("TRN2", target_bir_lowering=False, debug=False,
                   num_devices=ncores)

    def din(name, shape, dt=F32):
        return nc.dram_tensor(name, list(shape), dt, kind="ExternalInput").ap()

    # per-core tensors
    hT = din("hT", (D, NS))               # h_shard^T fp32 (residual stream)
    h8T = din("h8T", (D, NS), F8)         # h_shard^T fp8
    hN8 = din("hN8", (NS, D), F8)         # h_shard natural fp8
    bT = din("bT", (BIAS, NS), BF16)      # bias_feat^T shard
    # shared weights
    Wkp8 = din("Wkp8", (D, 64), F8)       # [64*Wkp | pad | ones@32 | pad]
    Wb = din("Wb", (BIAS, H), BF16)       # 64*Wb
    W1t8 = din("W1t8", (D, D), F8)        # 64*W1[:D]
    Wgt8 = din("Wgt8", (D, D), F8)        # 64*Wg[:D]
    W2h8 = din("W2h8", (D, D), F8)        # 64*RES*W2
    Wv8 = din("Wv8", (D, D), F8)          # 64*Wv
    Wo8 = din("Wo8", (D, D), F8)          # 64*Wo
    W1b8 = din("W1b8", (D, D), F8)        # 64*W1[D:]
    Wgb8 = din("Wgb8", (D, D), F8)        # 64*Wg[D:]
    # small constants
    idn = din("idn", (128, 128), F32)
    ones16 = din("ones16", (128, KT * 16 * NCH), F8)  # block c = ones in col c
    ncg1 = din("ncg1", (1, H), BF16)      # 64*(-cg) as K=1 stationary
    eps16 = din("eps16", (16, 1), F32)
    cbv = din("cbv", (H, 1), F32)         # cb per head (exp bias)
    hcvN = din("hcvN", (1, D), F32)       # h[c] natural row
    b1N = din("b1N", (1, D), F32)
    bgN = din("bgN", (1, D), F32)
    b2v = din("b2v", (128, KT), F32)      # 64*RES*b2

    outT = nc.dram_tensor("outT", [D, NS], F32, kind="ExternalOutput").ap()
    outC = nc.dram_tensor("outC", [1, D], F32, kind="ExternalOutput").ap()

    with tile.TileContext(nc) as tc:
        with (
            tc.tile_pool(name="persist", bufs=1) as pp,
            tc.tile_pool(name="dram", bufs=1, space="DRAM") as dram,
        ):
            # ---- long-lived small tiles ----
            idn_s = pp.tile([128, 128], F32, tag="idn")
            nc.sync.dma_start(out=idn_s[:], in_=idn[:])
            cbv_s = pp.tile([H, 1], F32, tag="cbv")
            nc.sync.dma_start(out=cbv_s[:], in_=cbv[:])
            b2v_s = pp.tile([128, KT], F32, tag="b2v")
            nc.sync.dma_start(out=b2v_s[:], in_=b2v[:])
            ones16_s = pp.tile([128, KT, 16 * NCH], F8, tag="ones16")
            nc.sync.dma_start(out=ones16_s[:], in_=ones16[:])
            ncg1_s = pp.tile([1, H], BF16, tag="ncg1")
            nc.sync.dma_start(out=ncg1_s[:], in_=ncg1[:])
            eps16_s = pp.tile([16, 1], F32, tag="eps16")
            nc.sync.dma_start(out=eps16_s[:], in_=eps16[:])
            Wkp8_s = pp.tile([128, KT, 64], F8, tag="Wkp8")
            for k in range(KT):
                nc.sync.dma_start(out=Wkp8_s[:, k:k + 1, :],
                                  in_=Wkp8[k * 128:(k + 1) * 128, :])
            Wb_s = pp.tile([BIAS, H], BF16, tag="Wb")
            nc.sync.dma_start(out=Wb_s[:], in_=Wb[:])

            h8_s = pp.tile([128, KT, NS], F8, tag="h8")
            Gacc = pp.tile([H, D], F32, tag="Gacc")
            sCols = pp.tile([H, NCH], F32, tag="sCols")
            g0_s = pp.tile([128, KT], F32, tag="g0")
            a0_s = pp.tile([128, KT], F32, tag="a0")
            GnT8 = pp.tile([128, KT, H], F8, tag="GnT8")
            ocv8 = pp.tile([128, KT, 16], F8, tag="ocv8")
            hcn8 = pp.tile([128, KT, 16], F8, tag="hcn8")

            # resident fp8 weights, loaded during pass 1 / the AR valley
            wres_cm = tc.tile_pool(name="wres", bufs=1)
            wres = wres_cm.__enter__()
            wv_s = wres.tile([128, KT, D], F8, tag="wv")
            wo_s = wres.tile([128, KT, D], F8, tag="wo")
            w1b_s = wres.tile([128, KT, D], F8, tag="w1b")
            wgb_s = wres.tile([128, KT, D], F8, tag="wgb")
            w1t_s = wres.tile([128, KT, D], F8, tag="w1t")
            wgt_s = wres.tile([128, KT, D], F8, tag="wgt")
            w2h_s = wres.tile([128, KT, D], F8, tag="w2h")
            PREFETCH = [(wv_s, Wv8), (wo_s, Wo8), (w1b_s, W1b8), (wgb_s, Wgb8)]
            VALLEY = [(w1t_s, W1t8), (wgt_s, Wgt8), (w2h_s, W2h8)]

            # =========================== PASS 1 ===========================
            # -- loop A: per-row sum and sum-of-squares, banked into a
            #    [16, CH] psum tile (chunk c -> row c via one-hot stationary)
            strips_cm = tc.tile_pool(name="strips", bufs=1)
            strips = strips_cm.__enter__()
            rb_all = strips.tile([1, NS], F32, tag="rb_all")  # 1/sd strip
            tm_all = strips.tile([1, NS], BF16, tag="tm_all")  # mean strip
            with (
                tc.tile_pool(name="pAsb", bufs=2) as sbA,
                tc.tile_pool(name="pAps", bufs=1, space="PSUM") as psS,
                tc.tile_pool(name="pAsb1", bufs=1) as sbM,
            ):
                SM16 = psS.tile([16, CH], F32, tag="SM16")
                SQ16 = psS.tile([16, CH], F32, tag="SQ16")
                # resident h8: 8 full k-slabs (8 KB/partition rows, max DMA
                # efficiency), alternating across the two hardware queues
                for k in range(KT):
                    eng = nc.sync if k % 2 == 0 else nc.scalar
                    eng.dma_start(
                        out=h8_s[:, k:k + 1, :],
                        in_=h8T[k * 128:(k + 1) * 128, :])
                for c in range(nch):
                    c0 = c * CH
                    if 1 <= c <= len(PREFETCH):
                        wsb, wd = PREFETCH[c - 1]
                        nc.scalar.dma_start(
                            out=wsb[:],
                            in_=wd[:].rearrange("(k p) j -> p k j", p=128))
                    sq8 = sbA.tile([128, KT, CH], F8, tag="sq8")
                    nc.vector.tensor_mul(sq8[:, 0:4, :],
                                         h8_s[:, 0:4, c0:c0 + CH],
                                         h8_s[:, 0:4, c0:c0 + CH])
                    nc.vector.tensor_mul(sq8[:, 4:8, :],
                                         h8_s[:, 4:8, c0:c0 + CH],
                                         h8_s[:, 4:8, c0:c0 + CH])
                    oc0 = c * 16
                    for kp in range(0, KT, 2):
                        nc.tensor.matmul(SM16[:],
                                         ones16_s[:, kp:kp + 2, oc0:oc0 + 16],
                                         h8_s[:, kp:kp + 2, c0:c0 + CH],
                                         start=(c == 0 and kp == 0),
                                         stop=(c == nch - 1 and kp == KT - 2),
                                         perf_mode=DRM)
                    for kp in range(0, KT, 2):
                        nc.tensor.matmul(SQ16[:],
                                         ones16_s[:, kp:kp + 2, oc0:oc0 + 16],
                                         sq8[:, kp:kp + 2, :],
                                         start=(c == 0 and kp == 0),
                                         stop=(c == nch - 1 and kp == KT - 2),
                                         perf_mode=DRM)
                # -- mid: batched LayerNorm stats for all 16 chunks at once
                tm16 = sbM.tile([16, CH], F32, tag="tm16")
                nc.vector.tensor_scalar_mul(tm16[:], SM16[:], 1.0 / D)
                msq16 = sbM.tile([16, CH], F32, tag="msq16")
                nc.vector.tensor_mul(msq16[:], tm16[:], tm16[:])
                var16 = sbM.tile([16, CH], F32, tag="var16")
                nc.vector.scalar_tensor_tensor(
                    var16[:], SQ16[:], 1.0 / D, msq16[:],
                    op0=OP.mult, op1=OP.subtract)
                sd16 = sbM.tile([16, CH], F32, tag="sd16")
                nc.scalar.activation(sd16[:], var16[:], AF.Sqrt,
                                     bias=eps16_s[:, 0:1])
                r16 = sbM.tile([16, CH], F32, tag="r16")
                nc.vector.reciprocal_approx_fast(r16[:], sd16[:])
                tmb16 = sbM.tile([16, CH], BF16, tag="tmb16")
                nc.vector.tensor_copy(tmb16[:], tm16[:])
                # reshape [16, CH] -> [1, NS] strips (row c -> cols c*CH...)
                nc.sync.dma_start(out=rb_all[:], in_=r16[:])
                nc.sync.dma_start(out=tm_all[:], in_=tmb16[:])

            # -- loop B: logits, softmax, and the G accumulation
            psG_cm = tc.tile_pool(name="psG", bufs=1, space="PSUM")
            psG = psG_cm.__enter__()
            G = psG.tile([H, D], F32, tag="G")
            with (
                tc.tile_pool(name="p1sb", bufs=1) as sb1,
                tc.tile_pool(name="p1sb2", bufs=2) as sb2,
                tc.tile_pool(name="p1psA", bufs=2, space="PSUM") as psA,
                tc.tile_pool(name="p1psB", bufs=1, space="PSUM") as psB,
            ):
                for c in range(nch):
                    c0 = c * CH
                    # pass-2 weights stream early so they are on-chip well
                    # before the AllReduce (concurrent bulk DMA slows it)
                    if c in (5, 9, 13):
                        wsb, wd = VALLEY[(c - 5) // 4]
                        nc.scalar.dma_start(
                            out=wsb[:],
                            in_=wd[:].rearrange("(k p) j -> p k j", p=128))
                    hN8c = sb2.tile([128, 4, D], F8, tag="hN8c")
                    nc.sync.dma_start(
                        out=hN8c[:],
                        in_=hN8[c0:c0 + CH, :].rearrange(
                            "(jj p) d -> p jj d", p=128))
                    bTc = sb2.tile([BIAS, CH], BF16, tag="bTc")
                    nc.sync.dma_start(out=bTc[:], in_=bT[:, c0:c0 + CH])

                    # Lp = 64*(Wkp^T h8 + ncg x m)  (ncg term via K=1 matmul)
                    Lp = psA.tile([H, CH], F32, tag="Lp")
                    for kp in range(0, KT, 2):
                        nc.tensor.matmul(Lp[:], Wkp8_s[:, kp:kp + 2, 0:16],
                                         h8_s[:, kp:kp + 2, c0:c0 + CH],
                                         start=(kp == 0), stop=False,
                                         perf_mode=DRM)
                    nc.tensor.matmul(Lp[:], ncg1_s[:],
                                     tm_all[:, c0:c0 + CH],
                                     start=False, stop=True)
                    L2 = psB.tile([H, CH], F32, tag="L2")
                    nc.tensor.matmul(L2[:], Wb_s[:], bTc[:],
                                     start=True, stop=True)

                    rb16 = sb2.tile([H, CH], F32, tag="rb16")
                    nc.gpsimd.partition_broadcast(rb16[:],
                                                  rb_all[:, c0:c0 + CH])
                    t3 = sb1.tile([H, CH], F32, tag="t3")
                    nc.vector.tensor_mul(t3[:], Lp[:], rb16[:])
                    t5 = sb2.tile([H, CH], F32, tag="t5")
                    nc.vector.tensor_add(t5[:], t3[:], L2[:])
                    pT = sb2.tile([H, CH], F32, tag="pT")
                    nc.scalar.activation(pT[:], t5[:], AF.Exp,
                                         bias=cbv_s[:, 0:1], scale=1.0 / SCL,
                                         accum_out=sCols[:, c:c + 1])
                    prT = sb2.tile([H, CH], F32, tag="prT")
                    nc.vector.tensor_mul(prT[:], pT[:], rb16[:])
                    # transpose p*r to natural fp8 and accumulate G
                    tp = psB.tile([128, 4 * H], F32, tag="tp")
                    for j in range(4):
                        nc.tensor.transpose(
                            tp[:, j * H:(j + 1) * H],
                            prT[:, j * 128:(j + 1) * 128],
                            idn_s[0:16, 0:16])
                    pr8 = sb2.tile([128, 4, H], F8, tag="pr8")
                    nc.vector.tensor_copy(pr8[:], tp[:])
                    for jp in (0, 2):
                        for half in range(2):
                            h0 = half * CH
                            nc.tensor.matmul(
                                G[:, h0:h0 + CH],
                                pr8[:, jp:jp + 2, :],
                                hN8c[:, jp:jp + 2, h0:h0 + CH],
                                start=(c == 0 and jp == 0),
                                stop=(c == nch - 1 and jp == 2),
                                perf_mode=DRM)
                nc.vector.tensor_copy(Gacc[:], G[:])
                if variant == "p1":
                    nc.sync.dma_start(out=outT[0:H, 0:D], in_=Gacc[:])
                    nc.sync.dma_start(out=outT[H:2 * H, 0:NCH], in_=sCols[:])
            strips_cm.__exit__(None, None, None)

            if variant != "p1":
                psG_cm.__exit__(None, None, None)
                # ---- local partials -> AllReduce ----
                # PRM = row-sum(G)/D exactly (sum_d G[h,d] = D * sum p*r*m)
                S16 = pp.tile([H, 1], F32, tag="S16")
                nc.vector.reduce_sum(S16[:], sCols[:], axis=AX.X)
                PRM16 = pp.tile([H, 1], F32, tag="PRM16")
                nc.vector.reduce_sum(PRM16[:], Gacc[:], axis=AX.X)
                nc.vector.tensor_scalar_mul(PRM16[:], PRM16[:], 1.0 / D)

                arin = dram.tile([H, D + 2], F32, tag="arin")
                arout = dram.tile([H, D + 2], F32, tag="arout")
                nc.sync.dma_start(out=arin[:, 0:D], in_=Gacc[:])
                nc.sync.dma_start(out=arin[:, D:D + 1], in_=PRM16[:])
                nc.sync.dma_start(out=arin[:, D + 1:D + 2], in_=S16[:])
                if variant == "nocc":
                    nc.sync.dma_start(out=arout[:], in_=arin[:])
                else:
                    nc.gpsimd.collective_compute(
                        "AllReduce", OP.add,
                        replica_groups=[list(range(ncores))],
                        ins=[arin.opt()], outs=[arout.opt()])
                # ---- G corrections + normalize + tiny GEMV chain ----
                with (
                    tc.tile_pool(name="postsb", bufs=1) as psb,
                    tc.tile_pool(name="postps", bufs=1, space="PSUM") as ps2,
                ):
                    hcv_s = psb.tile([1, D], F32, tag="hcv")
                    nc.sync.dma_start(out=hcv_s[:], in_=hcvN[:])
                    b1v_s = psb.tile([1, D], F32, tag="b1v")
                    nc.sync.dma_start(out=b1v_s[:], in_=b1N[:])
                    bgv_s = psb.tile([1, D], F32, tag="bgv")
                    nc.sync.dma_start(out=bgv_s[:], in_=bgN[:])
                    # gamma folded into Wv8, beta's affine term into hcvN
                    # (host) -> Gn = (Gar - PRM) / S in one pass
                    ARt = psb.tile([H, D + 2], F32, tag="ARt")
                    nc.sync.dma_start(out=ARt[:], in_=arout[:])
                    Gar = ARt[:, 0:D]
                    sr = psb.tile([H, 1], F32, tag="sr")
                    nc.vector.reciprocal(sr[:], ARt[:, D + 1:D + 2])
                    Gn = psb.tile([H, D], F32, tag="Gn")
                    nc.vector.tensor_scalar(Gn[:], Gar, ARt[:, D:D + 1],
                                            sr[:, 0:1],
                                            op0=OP.subtract, op1=OP.mult)

                    tpg = ps2.tile([128, KT * H], F32, tag="tpg")
                    for m in range(KT):
                        nc.tensor.transpose(
                            tpg[:, m * H:(m + 1) * H],
                            Gn[:, m * 128:(m + 1) * 128],
                            idn_s[0:16, 0:16])
                    nc.vector.tensor_copy(GnT8[:], tpg[:])

                    # out_center natural: Gn row h dot Wv columns (weights
                    # move, Gn^T stationary) -> [16, D] in halves
                    for half in range(2):
                        h0 = half * CH
                        OCh = ps2.tile([H, CH], F32, tag="OCh")
                        for kp in range(0, KT, 2):
                            nc.tensor.matmul(
                                OCh[:], GnT8[:, kp:kp + 2, :],
                                wv_s[:, kp:kp + 2, h0:h0 + CH],
                                start=(kp == 0), stop=(kp == KT - 2),
                                perf_mode=DRM)
                        OCsb = psb.tile([H, CH], F32, tag="OCsb")
                        nc.vector.tensor_copy(OCsb[:], OCh[:])
                        OCT = ps2.tile([128, 4 * H], F32, tag="OCT")
                        for kk in range(4):
                            k = half * 4 + kk
                            nc.tensor.transpose(
                                OCT[:, kk * H:(kk + 1) * H],
                                OCsb[:, kk * 128:(kk + 1) * 128],
                                idn_s[0:16, 0:16])
                            nc.vector.tensor_copy(
                                ocv8[0:64, k:k + 1, 0:1],
                                OCT[0:64, kk * H + 2 * k:kk * H + 2 * k + 1])
                            nc.vector.tensor_copy(
                                ocv8[64:128, k:k + 1, 0:1],
                                OCT[64:128,
                                    kk * H + 2 * k + 1:kk * H + 2 * k + 2])

                    # h_c_new natural row: ocv stationary, Wo moves
                    hcnN = psb.tile([1, D], F32, tag="hcnN")
                    for half in range(2):
                        h0 = half * CH
                        HCh = ps2.tile([1, CH], F32, tag="HCh")
                        for kp in range(0, KT, 2):
                            nc.tensor.matmul(
                                HCh[:], ocv8[:, kp:kp + 2, 0:1],
                                wo_s[:, kp:kp + 2, h0:h0 + CH],
                                start=(kp == 0), stop=(kp == KT - 2),
                                perf_mode=DRM)
                        nc.vector.scalar_tensor_tensor(
                            hcnN[:, h0:h0 + CH], HCh[:], RES / (SCL * SCL),
                            hcv_s[:, h0:h0 + CH], op0=OP.mult, op1=OP.add)
                    nc.sync.dma_start(out=outC[:], in_=hcnN[:])
                    # reshape [1, D] -> [128, KT] via PE transposes, cast fp8
                    hcT = ps2.tile([128, KT], F32, tag="vecT")
                    for k in range(KT):
                        nc.tensor.transpose(
                            hcT[:, k:k + 1],
                            hcnN[:, k * 128:(k + 1) * 128],
                            idn_s[0:1, 0:1])
                    nc.vector.tensor_copy(hcn8[:, :, 0:1], hcT[:])

                    # a0/g0 natural rows: hcn stationary, W1b/Wgb move
                    a0N = psb.tile([1, D], F32, tag="a0N")
                    g0N = psb.tile([1, D], F32, tag="g0N")
                    for dst, wsb, bias in ((a0N, w1b_s, b1v_s),
                                           (g0N, wgb_s, bgv_s)):
                        for half in range(2):
                            h0 = half * CH
                            A0h = ps2.tile([1, CH], F32, tag="A0h")
                            for kp in range(0, KT, 2):
                                nc.tensor.matmul(
                                    A0h[:], hcn8[:, kp:kp + 2, 0:1],
                                    wsb[:, kp:kp + 2, h0:h0 + CH],
                                    start=(kp == 0), stop=(kp == KT - 2),
                                    perf_mode=DRM)
                            nc.vector.scalar_tensor_tensor(
                                dst[:, h0:h0 + CH], A0h[:], 1.0 / SCL,
                                bias[:, h0:h0 + CH], op0=OP.mult, op1=OP.add)
                    for src, dst in ((a0N, a0_s), (g0N, g0_s)):
                        vT = ps2.tile([128, KT], F32, tag="vecT")
                        for k in range(KT):
                            nc.tensor.transpose(
                                vT[:, k:k + 1],
                                src[:, k * 128:(k + 1) * 128],
                                idn_s[0:1, 0:1])
                        nc.vector.tensor_copy(dst[:], vT[:])

            if variant in ("full", "nocc"):
                # =========================== PASS 2 ===========================
                with (
                    tc.tile_pool(name="p2sb", bufs=2) as sb3,
                    tc.tile_pool(name="p2st", bufs=3) as sb4,
                    tc.tile_pool(name="p2ps", bufs=2, space="PSUM") as ps3,
                ):
                    for c in range(NCH):
                        c0 = c * CH
                        hTrc = sb3.tile([128, KT, CH], F32, tag="hTrc")
                        nc.scalar.dma_start(
                            out=hTrc[:],
                            in_=hT[:, c0:c0 + CH].rearrange(
                                "(k p) j -> p k j", p=128))
                        B8 = sb3.tile([128, KT, CH], F8, tag="B8")
                        for m in range(KT):
                            A = ps3.tile([128, CH], F32, tag="A")
                            for kp in range(0, KT, 2):
                                nc.tensor.matmul(
                                    A[:],
                                    w1t_s[:, kp:kp + 2, m * 128:(m + 1) * 128],
                                    h8_s[:, kp:kp + 2, c0:c0 + CH],
                                    start=(kp == 0), stop=(kp == KT - 2),
                                    perf_mode=DRM)
                            # silu = z * sigmoid(z); SILU activations force an
                            # ACT table reload (1.28 us) on every call, so
                            # build it from SIGMOID (table-resident) + DVE
                            Az = sb4.tile([128, CH], F32, tag="Az")
                            nc.vector.tensor_scalar(
                                Az[:], A[:], 1.0 / SCL, a0_s[:, m:m + 1],
                                op0=OP.mult, op1=OP.add)
                            sg = sb4.tile([128, CH], F32, tag="sg")
                            nc.scalar.activation(sg[:], Az[:], AF.Sigmoid)
                            nc.vector.tensor_mul(B8[:, m:m + 1, :], Az[:],
                                                 sg[:])
                        for m in range(KT):
                            Gt = ps3.tile([128, CH], F32, tag="Gt")
                            for kp in range(0, KT, 2):
                                nc.tensor.matmul(
                                    Gt[:],
                                    wgt_s[:, kp:kp + 2, m * 128:(m + 1) * 128],
                                    h8_s[:, kp:kp + 2, c0:c0 + CH],
                                    start=(kp == 0), stop=(kp == KT - 2),
                                    perf_mode=DRM)
                            gs = sb4.tile([128, CH], F32, tag="gs")
                            nc.scalar.activation(gs[:], Gt[:], AF.Sigmoid,
                                                 bias=g0_s[:, m:m + 1],
                                                 scale=1.0 / SCL)
                            Cp = ps3.tile([128, CH], F32, tag="Cp")
                            for kp in range(0, KT, 2):
                                nc.tensor.matmul(
                                    Cp[:],
                                    w2h_s[:, kp:kp + 2, m * 128:(m + 1) * 128],
                                    B8[:, kp:kp + 2, :],
                                    start=(kp == 0), stop=(kp == KT - 2),
                                    perf_mode=DRM)
                            t6 = sb4.tile([128, CH], F32, tag="t6")
                            nc.vector.scalar_tensor_tensor(
                                t6[:], Cp[:], b2v_s[:, m:m + 1], gs[:],
                                op0=OP.add, op1=OP.mult)
                            nc.vector.scalar_tensor_tensor(
                                hTrc[:, m:m + 1, :], t6[:], 1.0 / SCL,
                                hTrc[:, m:m + 1, :],
                                op0=OP.mult, op1=OP.add)
                        nc.sync.dma_start(
                            out=outT[:, c0:c0 + CH].rearrange(
                                "(k p) j -> p k j", p=128),
                            in_=hTrc[:])
            wres_cm.__exit__(None, None, None)
    nc.compile()
    return nc


def _get_nc():
    if "nc" not in _CACHE:
        _CACHE["nc"] = _build()
    return _CACHE["nc"]


def kernel(h, center_idx, rbf_ic, seqsep_ic, nbr_idx, local_bias,
           gamma_c, beta_c, gamma_a, beta_a,
           Wq, Wk, Wv, Wo, Wb, W1, b1, W2, b2, Wg, bg):
    global LAST_RESULTS
    f = np.float32
    f8 = ml_dtypes.float8_e4m3
    bf = ml_dtypes.bfloat16
    h = np.asarray(h, f)
    c = int(center_idx)
    rbf_ic = np.asarray(rbf_ic, f)
    seqsep_ic = np.asarray(seqsep_ic, f)
    nbr_idx = np.asarray(nbr_idx)
    local_bias = np.asarray(local_bias, f)
    gamma_c = np.asarray(gamma_c, np.float64)
    beta_c = np.asarray(beta_c, np.float64)
    gamma_a = np.asarray(gamma_a, np.float64)
    beta_a = np.asarray(beta_a, np.float64)
    Wq = np.asarray(Wq, f); Wk = np.asarray(Wk, f); Wv = np.asarray(Wv, f)
    Wo = np.asarray(Wo, f); Wb = np.asarray(Wb, f)
    W1 = np.asarray(W1, f); b1 = np.asarray(b1, f)
    W2 = np.asarray(W2, f); b2 = np.asarray(b2, f)
    Wg = np.asarray(Wg, f); bg = np.asarray(bg, f)

    # ---- host algebra (tiny, no big matmuls) ----
    hc = h[c].astype(np.float64)
    hcl = (hc - hc.mean()) / np.sqrt(hc.var() + EPS) * gamma_c + beta_c
    q = (hcl @ Wq.astype(np.float64)).reshape(H, HD)
    Qm = np.zeros((D, H), np.float64)
    for hh in range(H):
        Qm[hh * HD:(hh + 1) * HD, hh] = q[hh] / np.sqrt(HD)
    Wk1 = Wk.astype(np.float64) @ Qm                    # (D, 16)
    Wkp = (Wk1 * gamma_a[:, None]).astype(f)
    ncg = (-(Wk1 * gamma_a[:, None]).sum(0)).astype(f).reshape(H, 1)
    cbv = (Wk1 * beta_a[:, None]).sum(0).astype(f).reshape(H, 1)

    Wkp8x = np.zeros((D, 64), f)
    Wkp8x[:, :H] = SCL * Wkp
    ones16x = np.zeros((128, KT, 16 * NCH), f)
    for cc in range(NCH):
        ones16x[:, :, cc * 16 + cc] = 1.0
    ones16x = ones16x.reshape(128, KT * 16 * NCH)

    full_bias = np.zeros((N, local_bias.shape[1]), f)
    full_bias[nbr_idx] = local_bias
    bias_featT = np.ascontiguousarray(
        np.concatenate([rbf_ic, seqsep_ic, full_bias], axis=1).T)  # (128, N)

    hT_full = np.ascontiguousarray(h.T)                 # (D, N)
    h8_full = h.astype(f8)                              # (N, D) fp8
    h8T_full = np.ascontiguousarray(h8_full.T)          # (D, N) fp8

    gamma_a32 = gamma_a.astype(f)
    beta_a32 = beta_a.astype(f)
    shared = {
        "Wkp8": Wkp8x.astype(f8), "Wb": (SCL * Wb).astype(bf),
        "W1t8": (SCL * np.ascontiguousarray(W1[:D])).astype(f8),
        "Wgt8": (SCL * np.ascontiguousarray(Wg[:D])).astype(f8),
        "W2h8": (SCL * RES * W2).astype(f8),
        "Wv8": (SCL * gamma_a[:, None] * Wv).astype(f8),
        "Wo8": (SCL * Wo).astype(f8),
        "W1b8": (SCL * np.ascontiguousarray(W1[D:])).astype(f8),
        "Wgb8": (SCL * np.ascontiguousarray(Wg[D:])).astype(f8),
        "idn": np.eye(128, dtype=f),
        "ones16": ones16x.astype(f8),
        "ncg1": (SCL * ncg).reshape(1, H).astype(bf),
        "eps16": np.full((16, 1), EPS, f),
        "cbv": cbv,
        "hcvN": (h[c] + RES * (Wo.astype(np.float64).T
                               @ (beta_a @ Wv.astype(np.float64)))
                 ).astype(f).reshape(1, D),
        "b1N": b1.reshape(1, D).copy(),
        "bgN": bg.reshape(1, D).copy(),
        "b2v": np.ascontiguousarray((SCL * RES * b2).reshape(KT, 128).T),
    }
    in_maps = []
    for i in range(NCORES):
        r0 = i * NS
        m = dict(shared)
        m["hT"] = np.ascontiguousarray(hT_full[:, r0:r0 + NS])
        m["h8T"] = np.ascontiguousarray(h8T_full[:, r0:r0 + NS])
        m["hN8"] = h8_full[r0:r0 + NS]
        m["bT"] = np.ascontiguousarray(bias_featT[:, r0:r0 + NS]).astype(bf)
        in_maps.append(m)

    nc = _get_nc()
    trace = bool(int(os.environ.get("KERNEL_TRACE", "0")))
    res = run_bass_kernel_spmd(nc, in_maps, core_ids=list(range(NCORES)),
                               trace=trace)
    LAST_RESULTS = res

    out = np.empty((N, D), f)
    for i in range(NCORES):
        out[i * NS:(i + 1) * NS] = res.results[i]["outT"].T
    out[c] = res.results[0]["outC"].reshape(D)
    return out


# revision 53
# speedup vs baseline: 1.1476x; 1.0395x over previous
"""Trainium2 Bass kernel for CenterGeoAttention (N=65536, D=1024, H=16).

Strategy (row-shard N across 8 cores, fp8 DoubleRow compute):

  - All heavy matmuls run in fp8e4m3 with MatmulPerfMode.DoubleRow
    (K=256 per instruction, 2 multiplies/cycle) against an SBUF-resident
    fp8 copy of the h shard (h8_s, 8 MiB).  Weights are pre-scaled by 64
    on the host so their N(0, 0.02) entries sit in fp8's normal range;
    the 1/64 descale folds into activation-scale / scalar_tensor_tensor.
  - LayerNorm is folded into rank-1 corrections (as before): the
    logits sweep's stationary operand carries [64*Wkp | ones] so row 16
    of the output is the per-row sum (mean) for free; the sumsq sweep
    uses fp8 squares computed on the Scalar engine.
  - The weighted V sum never materializes V: G = (p*r)^T @ h8 via fp8
    DoubleRow, AllReduce-add [G | PRM | S], then the tiny post-AR GEMV
    chain (out_center, h_c_new, a0, g0) runs on fp8 weights.
  - Pass 2: 3 big DR matmuls per chunk (h@W1t, h@Wgt, silu@W2) read the
    resident h8; fp32 h streams in only for the residual add.
  - Wv/Wo/W1b/Wgb prefetch during pass-1 chunks; W1t/Wgt/W2h load
    during the AllReduce so the inter-pass valley is just AR latency.
"""

import os
import ml_dtypes
import numpy as np

import concourse.bass as bass
import concourse.bacc as bacc
import concourse.tile as tile
import concourse.mybir as mybir
from concourse.bass_utils import run_bass_kernel_spmd

F32 = mybir.dt.float32
F8 = mybir.dt.float8e4
BF16 = mybir.dt.bfloat16
AF = mybir.ActivationFunctionType
OP = mybir.AluOpType
AX = mybir.AxisListType
DRM = mybir.MatmulPerfMode.DoubleRow

NCORES = 8
N, D, H, HD, BIAS = 65536, 1024, 16, 64, 128
NS = N // NCORES            # 8192 rows per core
CH = 512                    # row-chunk
NCH = NS // CH              # 16 chunks
KT = D // 128               # 8 feature tiles
EPS = 1e-5
RES = 0.5
SCL = 64.0                  # fp8 weight pre-scale

_CACHE = {}
LAST_RESULTS = None  # BassKernelResults from the most recent run (for test.py)


def _build(ncores=NCORES, variant="full", nch=NCH):
    nc = bacc.Bacc("TRN2", target_bir_lowering=False, debug=False,
                   num_devices=ncores)

    def din(name, shape, dt=F32):
        return nc.dram_tensor(name, list(shape), dt, kind="ExternalInput").ap()

    # per-core tensors
    hT = din("hT", (D, NS))               # h_shard^T fp32 (residual stream)
    h8T = din("h8T", (D, NS), F8)         # h_shard^T fp8
    hN8 = din("hN8", (NS, D), F8)         # h_shard natural fp8
    bT = din("bT", (BIAS, NS), BF16)      # bias_feat^T shard
    # shared weights
    Wkp8 = din("Wkp8", (D, 64), F8)       # [64*Wkp | pad | ones@32 | pad]
    Wb = din("Wb", (BIAS, H), BF16)       # 64*Wb
    W1t8 = din("W1t8", (D, D), F8)        # 64*W1[:D]
    Wgt8 = din("Wgt8", (D, D), F8)        # 64*Wg[:D]
    W2h8 = din("W2h8", (D, D), F8)        # 64*RES*W2
    Wv8 = din("Wv8", (D, D), F8)          # 64*Wv
    Wo8 = din("Wo8", (D, D), F8)          # 64*Wo
    W1b8 = din("W1b8", (D, D), F8)        # 64*W1[D:]
    Wgb8 = din("Wgb8", (D, D), F8)        # 64*Wg[D:]
    # small constants
    idn = din("idn", (128, 128), F32)
    ones16 = din("ones16", (128, KT * 16 * NCH), F8)  # block c = ones in col c
    ncg1 = din("ncg1", (1, H), BF16)      # 64*(-cg) as K=1 stationary
    eps16 = din("eps16", (16, 1), F32)
    cbv = din("cbv", (H, 1), F32)         # cb per head (exp bias)
    hcvN = din("hcvN", (1, D), F32)       # h[c] natural row
    b1N = din("b1N", (1, D), F32)
    bgN = din("bgN", (1, D), F32)
    b2v = din("b2v", (128, KT), F32)      # 64*RES*b2

    outT = nc.dram_tensor("outT", [D, NS], F32, kind="ExternalOutput").ap()
    outC = nc.dram_tensor("outC", [1, D], F32, kind="ExternalOutput").ap()

    with tile.TileContext(nc) as tc:
        with (
            tc.tile_pool(name="persist", bufs=1) as pp,
            tc.tile_pool(name="dram", bufs=1, space="DRAM") as dram,
        ):
            # ---- resident h8, first column-quarter before the constants so
            #      chunk 0 unblocks fast; rest after ----
            h8_s = pp.tile([128, KT, NS], F8, tag="h8")
            QW = NS // 4

            def _load_h8_quarter(q):
                for k in range(KT):
                    eng = nc.sync if (q * KT + k) % 2 == 0 else nc.scalar
                    eng.dma_start(
                        out=h8_s[:, k:k + 1, q * QW:(q + 1) * QW],
                        in_=h8T[k * 128:(k + 1) * 128, q * QW:(q + 1) * QW])

            _load_h8_quarter(0)
            # ---- long-lived small tiles ----
            idn_s = pp.tile([128, 128], F32, tag="idn")
            nc.sync.dma_start(out=idn_s[:], in_=idn[:])
            cbv_s = pp.tile([H, 1], F32, tag="cbv")
            nc.sync.dma_start(out=cbv_s[:], in_=cbv[:])
            b2v_s = pp.tile([128, KT], F32, tag="b2v")
            nc.sync.dma_start(out=b2v_s[:], in_=b2v[:])
            ones16_s = pp.tile([128, KT, 16 * NCH], F8, tag="ones16")
            nc.sync.dma_start(out=ones16_s[:], in_=ones16[:])
            ncg1_s = pp.tile([1, H], BF16, tag="ncg1")
            nc.sync.dma_start(out=ncg1_s[:], in_=ncg1[:])
            eps16_s = pp.tile([16, 1], F32, tag="eps16")
            nc.sync.dma_start(out=eps16_s[:], in_=eps16[:])
            Wkp8_s = pp.tile([128, KT, 64], F8, tag="Wkp8")
            for k in range(KT):
                nc.sync.dma_start(out=Wkp8_s[:, k:k + 1, :],
                                  in_=Wkp8[k * 128:(k + 1) * 128, :])
            Wb_s = pp.tile([BIAS, H], BF16, tag="Wb")
            nc.sync.dma_start(out=Wb_s[:], in_=Wb[:])
            for q in range(1, 4):
                _load_h8_quarter(q)

            Gacc = pp.tile([H, D], F32, tag="Gacc")
            sCols = pp.tile([H, NCH], F32, tag="sCols")
            g0_s = pp.tile([128, KT], F32, tag="g0")
            a0_s = pp.tile([128, KT], F32, tag="a0")
            GnT8 = pp.tile([128, KT, H], F8, tag="GnT8")
            ocv8 = pp.tile([128, KT, 16], F8, tag="ocv8")
            hcn8 = pp.tile([128, KT, 16], F8, tag="hcn8")

            # resident fp8 weights, loaded during pass 1 / the AR valley
            wres_cm = tc.tile_pool(name="wres", bufs=1)
            wres = wres_cm.__enter__()
            wv_s = wres.tile([128, KT, D], F8, tag="wv")
            wo_s = wres.tile([128, KT, D], F8, tag="wo")
            w1b_s = wres.tile([128, KT, D], F8, tag="w1b")
            wgb_s = wres.tile([128, KT, D], F8, tag="wgb")
            w1t_s = wres.tile([128, KT, D], F8, tag="w1t")
            wgt_s = wres.tile([128, KT, D], F8, tag="wgt")
            w2h_s = wres.tile([128, KT, D], F8, tag="w2h")
            PREFETCH = [(wv_s, Wv8), (wo_s, Wo8), (w1b_s, W1b8), (wgb_s, Wgb8)]
            VALLEY = [(w1t_s, W1t8), (wgt_s, Wgt8), (w2h_s, W2h8)]

            # =========================== PASS 1 ===========================
            # -- loop A: per-row sum and sum-of-squares, banked into a
            #    [16, CH] psum tile (chunk c -> row c via one-hot stationary)
            strips_cm = tc.tile_pool(name="strips", bufs=1)
            strips = strips_cm.__enter__()
            rb_all = strips.tile([1, NS], F32, tag="rb_all")  # 1/sd strip
            tm_all = strips.tile([1, NS], BF16, tag="tm_all")  # mean strip
            with (
                tc.tile_pool(name="pAsb", bufs=2) as sbA,
                tc.tile_pool(name="pAps", bufs=1, space="PSUM") as psS,
                tc.tile_pool(name="pAsb1", bufs=1) as sbM,
            ):
                SM16 = psS.tile([16, CH], F32, tag="SM16")
                SQ16 = psS.tile([16, CH], F32, tag="SQ16")
                for c in range(nch):
                    c0 = c * CH
                    if 1 <= c <= len(PREFETCH):
                        wsb, wd = PREFETCH[c - 1]
                        nc.scalar.dma_start(
                            out=wsb[:],
                            in_=wd[:].rearrange("(k p) j -> p k j", p=128))
                    sq8 = sbA.tile([128, KT, CH], F8, tag="sq8")
                    nc.vector.tensor_mul(sq8[:, 0:4, :],
                                         h8_s[:, 0:4, c0:c0 + CH],
                                         h8_s[:, 0:4, c0:c0 + CH])
                    nc.scalar.activation(sq8[:, 4:8, :],
                                         h8_s[:, 4:8, c0:c0 + CH], AF.Square)
                    oc0 = c * 16
                    for kp in range(0, KT, 2):
                        nc.tensor.matmul(SM16[:],
                                         ones16_s[:, kp:kp + 2, oc0:oc0 + 16],
                                         h8_s[:, kp:kp + 2, c0:c0 + CH],
                                         start=(c == 0 and kp == 0),
                                         stop=(c == nch - 1 and kp == KT - 2),
                                         perf_mode=DRM)
                    for kp in range(0, KT, 2):
                        nc.tensor.matmul(SQ16[:],
                                         ones16_s[:, kp:kp + 2, oc0:oc0 + 16],
                                         sq8[:, kp:kp + 2, :],
                                         start=(c == 0 and kp == 0),
                                         stop=(c == nch - 1 and kp == KT - 2),
                                         perf_mode=DRM)
                # -- mid: batched LayerNorm stats for all 16 chunks at once
                tm16 = sbM.tile([16, CH], F32, tag="tm16")
                nc.vector.tensor_scalar_mul(tm16[:], SM16[:], 1.0 / D)
                msq16 = sbM.tile([16, CH], F32, tag="msq16")
                nc.vector.tensor_mul(msq16[:], tm16[:], tm16[:])
                var16 = sbM.tile([16, CH], F32, tag="var16")
                nc.vector.scalar_tensor_tensor(
                    var16[:], SQ16[:], 1.0 / D, msq16[:],
                    op0=OP.mult, op1=OP.subtract)
                sd16 = sbM.tile([16, CH], F32, tag="sd16")
                nc.scalar.activation(sd16[:], var16[:], AF.Sqrt,
                                     bias=eps16_s[:, 0:1])
                r16 = sbM.tile([16, CH], F32, tag="r16")
                nc.vector.reciprocal_approx_fast(r16[:], sd16[:])
                tmb16 = sbM.tile([16, CH], BF16, tag="tmb16")
                nc.vector.tensor_copy(tmb16[:], tm16[:])
                # reshape [16, CH] -> [1, NS] strips (row c -> cols c*CH...)
                nc.sync.dma_start(out=rb_all[:], in_=r16[:])
                nc.sync.dma_start(out=tm_all[:], in_=tmb16[:])

            # -- loop B: logits, softmax, and the G accumulation
            psG_cm = tc.tile_pool(name="psG", bufs=1, space="PSUM")
            psG = psG_cm.__enter__()
            G = psG.tile([H, D], F32, tag="G")
            with (
                tc.tile_pool(name="p1sb", bufs=1) as sb1,
                tc.tile_pool(name="p1sb2", bufs=2) as sb2,
                tc.tile_pool(name="p1psA", bufs=2, space="PSUM") as psA,
                tc.tile_pool(name="p1psB", bufs=1, space="PSUM") as psB,
            ):
                for c in range(nch):
                    c0 = c * CH
                    # pass-2 weights stream early so they are on-chip well
                    # before the AllReduce (concurrent bulk DMA slows it)
                    if c in (5, 9, 13):
                        wsb, wd = VALLEY[(c - 5) // 4]
                        nc.scalar.dma_start(
                            out=wsb[:],
                            in_=wd[:].rearrange("(k p) j -> p k j", p=128))
                    hN8c = sb2.tile([128, 4, D], F8, tag="hN8c")
                    nc.sync.dma_start(
                        out=hN8c[:],
                        in_=hN8[c0:c0 + CH, :].rearrange(
                            "(jj p) d -> p jj d", p=128))
                    bTc = sb2.tile([BIAS, CH], BF16, tag="bTc")
                    nc.sync.dma_start(out=bTc[:], in_=bT[:, c0:c0 + CH])

                    # Lp = 64*(Wkp^T h8 + ncg x m)  (ncg term via K=1 matmul)
                    Lp = psA.tile([H, CH], F32, tag="Lp")
                    for kp in range(0, KT, 2):
                        nc.tensor.matmul(Lp[:], Wkp8_s[:, kp:kp + 2, 0:16],
                                         h8_s[:, kp:kp + 2, c0:c0 + CH],
                                         start=(kp == 0), stop=False,
                                         perf_mode=DRM)
                    nc.tensor.matmul(Lp[:], ncg1_s[:],
                                     tm_all[:, c0:c0 + CH],
                                     start=False, stop=True)
                    L2 = psB.tile([H, CH], F32, tag="L2")
                    nc.tensor.matmul(L2[:], Wb_s[:], bTc[:],
                                     start=True, stop=True)

                    rb16 = sb2.tile([H, CH], F32, tag="rb16")
                    nc.gpsimd.partition_broadcast(rb16[:],
                                                  rb_all[:, c0:c0 + CH])
                    t3 = sb1.tile([H, CH], F32, tag="t3")
                    nc.vector.tensor_mul(t3[:], Lp[:], rb16[:])
                    t5 = sb2.tile([H, CH], F32, tag="t5")
                    nc.vector.tensor_add(t5[:], t3[:], L2[:])
                    pT = sb2.tile([H, CH], F32, tag="pT")
                    nc.scalar.activation(pT[:], t5[:], AF.Exp,
                                         bias=cbv_s[:, 0:1], scale=1.0 / SCL,
                                         accum_out=sCols[:, c:c + 1])
                    prT = sb2.tile([H, CH], F32, tag="prT")
                    nc.vector.tensor_mul(prT[:], pT[:], rb16[:])
                    # transpose p*r to natural fp8 and accumulate G
                    tp = psB.tile([128, 4 * H], F32, tag="tp")
                    for j in range(4):
                        nc.tensor.transpose(
                            tp[:, j * H:(j + 1) * H],
                            prT[:, j * 128:(j + 1) * 128],
                            idn_s[0:16, 0:16])
                    pr8 = sb2.tile([128, 4, H], F8, tag="pr8")
                    nc.vector.tensor_copy(pr8[:], tp[:])
                    for jp in (0, 2):
                        for half in range(2):
                            h0 = half * CH
                            nc.tensor.matmul(
                                G[:, h0:h0 + CH],
                                pr8[:, jp:jp + 2, :],
                                hN8c[:, jp:jp + 2, h0:h0 + CH],
                                start=(c == 0 and jp == 0),
                                stop=(c == nch - 1 and jp == 2),
                                perf_mode=DRM)
                nc.vector.tensor_copy(Gacc[:], G[:])
                if variant == "p1":
                    nc.sync.dma_start(out=outT[0:H, 0:D], in_=Gacc[:])
                    nc.sync.dma_start(out=outT[H:2 * H, 0:NCH], in_=sCols[:])
            strips_cm.__exit__(None, None, None)

            if variant != "p1":
                psG_cm.__exit__(None, None, None)
                # ---- local partials -> AllReduce ----
                # PRM = row-sum(G)/D exactly (sum_d G[h,d] = D * sum p*r*m)
                S16 = pp.tile([H, 1], F32, tag="S16")
                nc.vector.reduce_sum(S16[:], sCols[:], axis=AX.X)
                PRM16 = pp.tile([H, 1], F32, tag="PRM16")
                nc.vector.reduce_sum(PRM16[:], Gacc[:], axis=AX.X)
                nc.vector.tensor_scalar_mul(PRM16[:], PRM16[:], 1.0 / D)

                arin = dram.tile([H, D + 2], F32, tag="arin")
                arout = dram.tile([H, D + 2], F32, tag="arout")
                nc.sync.dma_start(out=arin[:, 0:D], in_=Gacc[:])
                nc.sync.dma_start(out=arin[:, D:D + 1], in_=PRM16[:])
                nc.sync.dma_start(out=arin[:, D + 1:D + 2], in_=S16[:])
                if variant == "nocc":
                    nc.sync.dma_start(out=arout[:], in_=arin[:])
                else:
                    nc.gpsimd.collective_compute(
                        "AllReduce", OP.add,
                        replica_groups=[list(range(ncores))],
                        ins=[arin.opt()], outs=[arout.opt()])
                # ---- G corrections + normalize + tiny GEMV chain ----
                with (
                    tc.tile_pool(name="postsb", bufs=1) as psb,
                    tc.tile_pool(name="postps", bufs=1, space="PSUM") as ps2,
                ):
                    hcv_s = psb.tile([1, D], F32, tag="hcv")
                    nc.sync.dma_start(out=hcv_s[:], in_=hcvN[:])
                    b1v_s = psb.tile([1, D], F32, tag="b1v")
                    nc.sync.dma_start(out=b1v_s[:], in_=b1N[:])
                    bgv_s = psb.tile([1, D], F32, tag="bgv")
                    nc.sync.dma_start(out=bgv_s[:], in_=bgN[:])
                    # gamma folded into Wv8, beta's affine term into hcvN
                    # (host) -> Gn = (Gar - PRM) / S in one pass
                    ARt = psb.tile([H, D + 2], F32, tag="ARt")
                    nc.sync.dma_start(out=ARt[:], in_=arout[:])
                    Gar = ARt[:, 0:D]
                    sr = psb.tile([H, 1], F32, tag="sr")
                    nc.vector.reciprocal(sr[:], ARt[:, D + 1:D + 2])
                    Gn = psb.tile([H, D], F32, tag="Gn")
                    nc.vector.tensor_scalar(Gn[:], Gar, ARt[:, D:D + 1],
                                            sr[:, 0:1],
                                            op0=OP.subtract, op1=OP.mult)

                    tpg = ps2.tile([128, KT * H], F32, tag="tpg")
                    for m in range(KT):
                        nc.tensor.transpose(
                            tpg[:, m * H:(m + 1) * H],
                            Gn[:, m * 128:(m + 1) * 128],
                            idn_s[0:16, 0:16])
                    nc.vector.tensor_copy(GnT8[:], tpg[:])

                    # out_center natural: Gn row h dot Wv columns (weights
                    # move, Gn^T stationary) -> [16, D] in halves
                    for half in range(2):
                        h0 = half * CH
                        OCh = ps2.tile([H, CH], F32, tag="OCh")
                        for kp in range(0, KT, 2):
                            nc.tensor.matmul(
                                OCh[:], GnT8[:, kp:kp + 2, :],
                                wv_s[:, kp:kp + 2, h0:h0 + CH],
                                start=(kp == 0), stop=(kp == KT - 2),
                                perf_mode=DRM)
                        OCsb = psb.tile([H, CH], F32, tag="OCsb")
                        nc.vector.tensor_copy(OCsb[:], OCh[:])
                        OCT = ps2.tile([128, 4 * H], F32, tag="OCT")
                        for kk in range(4):
                            k = half * 4 + kk
                            nc.tensor.transpose(
                                OCT[:, kk * H:(kk + 1) * H],
                                OCsb[:, kk * 128:(kk + 1) * 128],
                                idn_s[0:16, 0:16])
                            nc.vector.tensor_copy(
                                ocv8[0:64, k:k + 1, 0:1],
                                OCT[0:64, kk * H + 2 * k:kk * H + 2 * k + 1])
                            nc.vector.tensor_copy(
                                ocv8[64:128, k:k + 1, 0:1],
                                OCT[64:128,
                                    kk * H + 2 * k + 1:kk * H + 2 * k + 2])

                    # h_c_new natural row: ocv stationary, Wo moves
                    hcnN = psb.tile([1, D], F32, tag="hcnN")
                    for half in range(2):
                        h0 = half * CH
                        HCh = ps2.tile([1, CH], F32, tag="HCh")
                        for kp in range(0, KT, 2):
                            nc.tensor.matmul(
                                HCh[:], ocv8[:, kp:kp + 2, 0:1],
                                wo_s[:, kp:kp + 2, h0:h0 + CH],
                                start=(kp == 0), stop=(kp == KT - 2),
                                perf_mode=DRM)
                        nc.vector.scalar_tensor_tensor(
                            hcnN[:, h0:h0 + CH], HCh[:], RES / (SCL * SCL),
                            hcv_s[:, h0:h0 + CH], op0=OP.mult, op1=OP.add)
                    nc.sync.dma_start(out=outC[:], in_=hcnN[:])
                    # reshape [1, D] -> [128, KT] via PE transposes, cast fp8
                    hcT = ps2.tile([128, KT], F32, tag="vecT")
                    for k in range(KT):
                        nc.tensor.transpose(
                            hcT[:, k:k + 1],
                            hcnN[:, k * 128:(k + 1) * 128],
                            idn_s[0:1, 0:1])
                    nc.vector.tensor_copy(hcn8[:, :, 0:1], hcT[:])

                    # a0/g0 natural rows: hcn stationary, W1b/Wgb move
                    a0N = psb.tile([1, D], F32, tag="a0N")
                    g0N = psb.tile([1, D], F32, tag="g0N")
                    for dst, wsb, bias in ((a0N, w1b_s, b1v_s),
                                           (g0N, wgb_s, bgv_s)):
                        for half in range(2):
                            h0 = half * CH
                            A0h = ps2.tile([1, CH], F32, tag="A0h")
                            for kp in range(0, KT, 2):
                                nc.tensor.matmul(
                                    A0h[:], hcn8[:, kp:kp + 2, 0:1],
                                    wsb[:, kp:kp + 2, h0:h0 + CH],
                                    start=(kp == 0), stop=(kp == KT - 2),
                                    perf_mode=DRM)
                            nc.vector.scalar_tensor_tensor(
                                dst[:, h0:h0 + CH], A0h[:], 1.0 / SCL,
                                bias[:, h0:h0 + CH], op0=OP.mult, op1=OP.add)
                    for src, dst in ((a0N, a0_s), (g0N, g0_s)):
                        vT = ps2.tile([128, KT], F32, tag="vecT")
                        for k in range(KT):
                            nc.tensor.transpose(
                                vT[:, k:k + 1],
                                src[:, k * 128:(k + 1) * 128],
                                idn_s[0:1, 0:1])
                        nc.vector.tensor_copy(dst[:], vT[:])

            if variant in ("full", "nocc"):
                # =========================== PASS 2 ===========================
                with (
                    tc.tile_pool(name="p2sb", bufs=2) as sb3,
                    tc.tile_pool(name="p2st", bufs=3) as sb4,
                    tc.tile_pool(name="p2ps", bufs=2, space="PSUM") as ps3,
                    tc.tile_pool(name="p2psA", bufs=3, space="PSUM") as ps3a,
                ):
                    for c in range(NCH):
                        c0 = c * CH
                        hTrc = sb3.tile([128, KT, CH], F32, tag="hTrc")
                        nc.scalar.dma_start(
                            out=hTrc[:],
                            in_=hT[:, c0:c0 + CH].rearrange(
                                "(k p) j -> p k j", p=128))
                        B8 = sb3.tile([128, KT, CH], F8, tag="B8")
                        for m in range(KT):
                            A = ps3a.tile([128, CH], F32, tag="A")
                            for kp in range(0, KT, 2):
                                nc.tensor.matmul(
                                    A[:],
                                    w1t_s[:, kp:kp + 2, m * 128:(m + 1) * 128],
                                    h8_s[:, kp:kp + 2, c0:c0 + CH],
                                    start=(kp == 0), stop=(kp == KT - 2),
                                    perf_mode=DRM)
                            # silu = z * sigmoid(z); SILU activations force an
                            # ACT table reload (1.28 us) on every call, so
                            # build it from SIGMOID (table-resident) + DVE
                            Az = sb4.tile([128, CH], F32, tag="Az")
                            nc.vector.tensor_scalar(
                                Az[:], A[:], 1.0 / SCL, a0_s[:, m:m + 1],
                                op0=OP.mult, op1=OP.add)
                            sg = sb4.tile([128, CH], F32, tag="sg")
                            nc.scalar.activation(sg[:], Az[:], AF.Sigmoid)
                            nc.vector.tensor_mul(B8[:, m:m + 1, :], Az[:],
                                                 sg[:])
                        for m in range(KT):
                            Gt = ps3.tile([128, CH], F32, tag="Gt")
                            for kp in range(0, KT, 2):
                                nc.tensor.matmul(
                                    Gt[:],
                                    wgt_s[:, kp:kp + 2, m * 128:(m + 1) * 128],
                                    h8_s[:, kp:kp + 2, c0:c0 + CH],
                                    start=(kp == 0), stop=(kp == KT - 2),
                                    perf_mode=DRM)
                            gs = sb4.tile([128, CH], F32, tag="gs")
                            nc.scalar.activation(gs[:], Gt[:], AF.Sigmoid,
                                                 bias=g0_s[:, m:m + 1],
                                                 scale=1.0 / SCL)
                            Cp = ps3.tile([128, CH], F32, tag="Cp")
                            for kp in range(0, KT, 2):
                                nc.tensor.matmul(
                                    Cp[:],
                                    w2h_s[:, kp:kp + 2, m * 128:(m + 1) * 128],
                                    B8[:, kp:kp + 2, :],
                                    start=(kp == 0), stop=(kp == KT - 2),
                                    perf_mode=DRM)
                            t6 = sb4.tile([128, CH], F32, tag="t6")
                            nc.vector.scalar_tensor_tensor(
                                t6[:], Cp[:], b2v_s[:, m:m + 1], gs[:],
                                op0=OP.add, op1=OP.mult)
                            nc.vector.scalar_tensor_tensor(
                                hTrc[:, m:m + 1, :], t6[:], 1.0 / SCL,
                                hTrc[:, m:m + 1, :],
                                op0=OP.mult, op1=OP.add)
                        nc.sync.dma_start(
                            out=outT[:, c0:c0 + CH].rearrange(
                                "(k p) j -> p k j", p=128),
                            in_=hTrc[:])
            wres_cm.__exit__(None, None, None)
    nc.compile()
    return nc


def _get_nc():
    if "nc" not in _CACHE:
        _CACHE["nc"] = _build()
    return _CACHE["nc"]


def kernel(h, center_idx, rbf_ic, seqsep_ic, nbr_idx, local_bias,
           gamma_c, beta_c, gamma_a, beta_a,
           Wq, Wk, Wv, Wo, Wb, W1, b1, W2, b2, Wg, bg):
    global LAST_RESULTS
    f = np.float32
    f8 = ml_dtypes.float8_e4m3
    bf = ml_dtypes.bfloat16
    h = np.asarray(h, f)
    c = int(center_idx)
    rbf_ic = np.asarray(rbf_ic, f)
    seqsep_ic = np.asarray(seqsep_ic, f)
    nbr_idx = np.asarray(nbr_idx)
    local_bias = np.asarray(local_bias, f)
    gamma_c = np.asarray(gamma_c, np.float64)
    beta_c = np.asarray(beta_c, np.float64)
    gamma_a = np.asarray(gamma_a, np.float64)
    beta_a = np.asarray(beta_a, np.float64)
    Wq = np.asarray(Wq, f); Wk = np.asarray(Wk, f); Wv = np.asarray(Wv, f)
    Wo = np.asarray(Wo, f); Wb = np.asarray(Wb, f)
    W1 = np.asarray(W1, f); b1 = np.asarray(b1, f)
    W2 = np.asarray(W2, f); b2 = np.asarray(b2, f)
    Wg = np.asarray(Wg, f); bg = np.asarray(bg, f)

    # ---- host algebra (tiny, no big matmuls) ----
    hc = h[c].astype(np.float64)
    hcl = (hc - hc.mean()) / np.sqrt(hc.var() + EPS) * gamma_c + beta_c
    q = (hcl @ Wq.astype(np.float64)).reshape(H, HD)
    Qm = np.zeros((D, H), np.float64)
    for hh in range(H):
        Qm[hh * HD:(hh + 1) * HD, hh] = q[hh] / np.sqrt(HD)
    Wk1 = Wk.astype(np.float64) @ Qm                    # (D, 16)
    Wkp = (Wk1 * gamma_a[:, None]).astype(f)
    ncg = (-(Wk1 * gamma_a[:, None]).sum(0)).astype(f).reshape(H, 1)
    cbv = (Wk1 * beta_a[:, None]).sum(0).astype(f).reshape(H, 1)

    Wkp8x = np.zeros((D, 64), f)
    Wkp8x[:, :H] = SCL * Wkp
    ones16x = np.zeros((128, KT, 16 * NCH), f)
    for cc in range(NCH):
        ones16x[:, :, cc * 16 + cc] = 1.0
    ones16x = ones16x.reshape(128, KT * 16 * NCH)

    full_bias = np.zeros((N, local_bias.shape[1]), f)
    full_bias[nbr_idx] = local_bias
    bias_featT = np.ascontiguousarray(
        np.concatenate([rbf_ic, seqsep_ic, full_bias], axis=1).T)  # (128, N)

    hT_full = np.ascontiguousarray(h.T)                 # (D, N)
    h8_full = h.astype(f8)                              # (N, D) fp8
    h8T_full = np.ascontiguousarray(h8_full.T)          # (D, N) fp8

    gamma_a32 = gamma_a.astype(f)
    beta_a32 = beta_a.astype(f)
    shared = {
        "Wkp8": Wkp8x.astype(f8), "Wb": (SCL * Wb).astype(bf),
        "W1t8": (SCL * np.ascontiguousarray(W1[:D])).astype(f8),
        "Wgt8": (SCL * np.ascontiguousarray(Wg[:D])).astype(f8),
        "W2h8": (SCL * RES * W2).astype(f8),
        "Wv8": (SCL * gamma_a[:, None] * Wv).astype(f8),
        "Wo8": (SCL * Wo).astype(f8),
        "W1b8": (SCL * np.ascontiguousarray(W1[D:])).astype(f8),
        "Wgb8": (SCL * np.ascontiguousarray(Wg[D:])).astype(f8),
        "idn": np.eye(128, dtype=f),
        "ones16": ones16x.astype(f8),
        "ncg1": (SCL * ncg).reshape(1, H).astype(bf),
        "eps16": np.full((16, 1), EPS, f),
        "cbv": cbv,
        "hcvN": (h[c] + RES * (Wo.astype(np.float64).T
                               @ (beta_a @ Wv.astype(np.float64)))
                 ).astype(f).reshape(1, D),
        "b1N": b1.reshape(1, D).copy(),
        "bgN": bg.reshape(1, D).copy(),
        "b2v": np.ascontiguousarray((SCL * RES * b2).reshape(KT, 128).T),
    }
    in_maps = []
    for i in range(NCORES):
        r0 = i * NS
        m = dict(shared)
        m["hT"] = np.ascontiguousarray(hT_full[:, r0:r0 + NS])
        m["h8T"] = np.ascontiguousarray(h8T_full[:, r0:r0 + NS])
        m["hN8"] = h8_full[r0:r0 + NS]
        m["bT"] = np.ascontiguousarray(bias_featT[:, r0:r0 + NS]).astype(bf)
        in_maps.append(m)

    nc = _get_nc()
    trace = bool(int(os.environ.get("KERNEL_TRACE", "0")))
    res = run_bass_kernel_spmd(nc, in_maps, core_ids=list(range(NCORES)),
                               trace=trace)
    LAST_RESULTS = res

    out = np.empty((N, D), f)
    for i in range(NCORES):
        out[i * NS:(i + 1) * NS] = res.results[i]["outT"].T
    out[c] = res.results[0]["outC"].reshape(D)
    return out
